# revision 1
# baseline (speedup 1.0000x reference)
"""Trainium2 Bass kernel for nn_AttentionLayer_85383949844589.

Gated attention layer: B=16, C=K=128, D=256.
  g0 = BN0(q @ W0.T)          per-C-channel stats over (B, D)
  g1 = BN1(kc @ W1.T)         per-K-channel stats over (B, D)
  aw[b,c,k,d]   = sigmoid(g1)[b,k,d] * sigmoid(g0)[b,c,d]
  attn[b,c,k,d] = kc[b,k,d] * aw * cmask[b,c] * kmask[b,k]
  out[b,c,d]    = tanh(sum_k attn / klen[b])
  awm[b,c,k]    = mean_d aw

Sharding: the C (query-channel) axis is split across the 8 NeuronCores
(16 channels each).  BN0 stats are per-C-channel, so they are fully local
to a core; the g1/BN1 pipeline is replicated on every core (it is tiny).
No cross-core communication is needed at all.

Per core the dominant cost is writing its (B, C/8, K, D) = 32 MiB slice of
attn, i.e. the kernel is HBM-write-bound (~95 us at ~358 GB/s/core).

Device layouts (host pre-packs everything into DMA-friendly layouts):
  Qg[b]  = sigmoid(g0_local[b]) * cmask      (16 part,  256 free)
  A[b]   = kc[b] * sigmoid(g1[b]) * kmask    (128 part, 256 free)
  attn[b, c, k, :] = Qg[b,c,:] * A[b,k,:]
The row-broadcast of Qg over the 128 k-partitions is done on the PE with a
ones(1,128) stationary matmul into PSUM; the DVE multiplies PSUM by A.
"""

import sys

sys.path.insert(0, "/opt/trn_rl_repo")

import numpy as np

B, C, K, D = 16, 128, 128, 256
NCORES = 8
CL = C // NCORES  # 16 query channels per core
EPS = 1e-5

_CACHE: dict = {}


def _build_nc():
    import concourse.tile as tile
    from concourse import bacc, mybir

    fp32 = mybir.dt.float32
    AF = mybir.ActivationFunctionType
    OP = mybir.AluOpType
    AX = mybir.AxisListType

    nc = bacc.Bacc(trn_type="TRN2", debug=False, num_devices=NCORES)

    # ---- DRAM I/O ----
    # qt[p, b, h, c]  = q[b, c_slice[c], h*128+p]
    qt_d = nc.dram_tensor("qt", [128, B, 2, CL], fp32, kind="ExternalInput")
    # kcn[k, b, d]    = kc[b, k, d]
    kcn_d = nc.dram_tensor("kcn", [K, B, D], fp32, kind="ExternalInput")
    # kct[p, b, h, k] = kc[b, k, h*128+p]
    kct_d = nc.dram_tensor("kct", [128, B, 2, K], fp32, kind="ExternalInput")
    # wXt[p, h, o]    = WX[o, h*128+p]
    w0t_d = nc.dram_tensor("w0t", [128, 2, D], fp32, kind="ExternalInput")
    w1t_d = nc.dram_tensor("w1t", [128, 2, D], fp32, kind="ExternalInput")
    g0_d = nc.dram_tensor("g0", [CL, 1], fp32, kind="ExternalInput")
    b0_d = nc.dram_tensor("b0", [CL, 1], fp32, kind="ExternalInput")
    g1_d = nc.dram_tensor("g1", [K, 1], fp32, kind="ExternalInput")
    b1_d = nc.dram_tensor("b1", [K, 1], fp32, kind="ExternalInput")
    cmt_d = nc.dram_tensor("cmt", [CL, B], fp32, kind="ExternalInput")  # cmask.T
    kmt_d = nc.dram_tensor("kmt", [K, B], fp32, kind="ExternalInput")  # kmask.T
    il_d = nc.dram_tensor("il", [CL, B], fp32, kind="ExternalInput")  # 1/klen
    iden_d = nc.dram_tensor("iden", [128, 128], fp32, kind="ExternalInput")
    onc_d = nc.dram_tensor("onc", [128, CL], fp32, kind="ExternalInput")
    onr_d = nc.dram_tensor("onr", [1, 128], fp32, kind="ExternalInput")

    ores_d = nc.dram_tensor("o_res", [B, CL, D], fp32, kind="ExternalOutput")
    attn_d = nc.dram_tensor("o_attn", [B, CL, K, D], fp32, kind="ExternalOutput")
    awm_d = nc.dram_tensor("o_awm", [B, CL, K], fp32, kind="ExternalOutput")

    BD = float(B * D)

    with tile.TileContext(nc) as tc:
        with (
            tc.tile_pool(name="const", bufs=1) as cp,
            tc.tile_pool(name="persist", bufs=1) as pp,
            tc.tile_pool(name="stats", bufs=1) as sp,
            tc.tile_pool(name="work", bufs=2) as wp,
            tc.tile_pool(name="bigout", bufs=3) as bp,
        ):
            # ---- load constants / persistent inputs ----
            w0t = cp.tile([128, 2 * D], fp32)
            nc.sync.dma_start(w0t[:], w0t_d.ap().rearrange("p h o -> p (h o)"))
            w1t = cp.tile([128, 2 * D], fp32)
            nc.sync.dma_start(w1t[:], w1t_d.ap().rearrange("p h o -> p (h o)"))
            iden = cp.tile([128, 128], fp32)
            nc.sync.dma_start(iden[:], iden_d.ap()[:])
            onc = cp.tile([128, CL], fp32)
            nc.sync.dma_start(onc[:], onc_d.ap()[:])
            onr = cp.tile([1, 128], fp32)
            nc.sync.dma_start(onr[:], onr_d.ap()[:])
            g0c = cp.tile([CL, 1], fp32)
            nc.sync.dma_start(g0c[:], g0_d.ap()[:])
            b0c = cp.tile([CL, 1], fp32)
            nc.sync.dma_start(b0c[:], b0_d.ap()[:])
            g1c = cp.tile([K, 1], fp32)
            nc.sync.dma_start(g1c[:], g1_d.ap()[:])
            b1c = cp.tile([K, 1], fp32)
            nc.sync.dma_start(b1c[:], b1_d.ap()[:])
            cmt = cp.tile([CL, B], fp32)
            nc.sync.dma_start(cmt[:], cmt_d.ap()[:])
            kmt = cp.tile([K, B], fp32)
            nc.sync.dma_start(kmt[:], kmt_d.ap()[:])
            ilen = cp.tile([CL, B], fp32)
            nc.sync.dma_start(ilen[:], il_d.ap()[:])

            qt = pp.tile([128, B * 2 * CL], fp32)
            nc.sync.dma_start(qt[:], qt_d.ap().rearrange("p b h c -> p (b h c)"))
            kct = pp.tile([128, B * 2 * K], fp32)
            nc.sync.dma_start(kct[:], kct_d.ap().rearrange("p b h k -> p (b h k)"))
            kcn = pp.tile([K, B * D], fp32)
            nc.sync.dma_start(kcn[:], kcn_d.ap().rearrange("k b d -> k (b d)"))

            y1sb = pp.tile([K, B * D], fp32)
            y0sb = pp.tile([CL, B * D], fp32)

            s1cols = sp.tile([K, B], fp32)
            q1cols = sp.tile([K, B], fp32)
            s0cols = sp.tile([CL, B], fp32)
            q0cols = sp.tile([CL, B], fp32)
            sq1s = sp.tile([K, D], fp32)
            sq0s = sp.tile([CL, D], fp32)

            # ---- phase 1: Y0/Y1 matmuls + per-channel sum / sumsq ----
            with tc.tile_pool(name="ps1", bufs=2, space="PSUM") as ps1:
                for b in range(B):
                    y1ps = ps1.tile([K, D], fp32, tag="y1ps")
                    for h in range(2):
                        nc.tensor.matmul(
                            y1ps[:],
                            kct[:, b * 256 + h * 128 : b * 256 + h * 128 + 128],
                            w1t[:, h * D : (h + 1) * D],
                            start=(h == 0),
                            stop=(h == 1),
                        )
                    nc.scalar.activation(
                        y1sb[:, b * D : (b + 1) * D],
                        y1ps[:],
                        AF.Identity,
                        accum_out=s1cols[:, b : b + 1],
                    )
                    nc.scalar.activation(
                        sq1s[:], y1ps[:], AF.Square, accum_out=q1cols[:, b : b + 1]
                    )

                    y0ps = ps1.tile([CL, D], fp32, tag="y0ps")
                    for h in range(2):
                        nc.tensor.matmul(
                            y0ps[:],
                            qt[:, b * 2 * CL + h * CL : b * 2 * CL + (h + 1) * CL],
                            w0t[:, h * D : (h + 1) * D],
                            start=(h == 0),
                            stop=(h == 1),
                        )
                    nc.scalar.activation(
                        y0sb[:, b * D : (b + 1) * D],
                        y0ps[:],
                        AF.Identity,
                        accum_out=s0cols[:, b : b + 1],
                    )
                    nc.scalar.activation(
                        sq0s[:], y0ps[:], AF.Square, accum_out=q0cols[:, b : b + 1]
                    )

            # ---- phase boundary: finalize BN scale/shift ----
            # s = gamma / sqrt(var+eps);  t = beta - mean * s
            def bn_finalize(P, scols, qcols, gc, bc):
                ssum = sp.tile([P, 1], fp32, name=f"ssum{P}")
                nc.vector.tensor_reduce(ssum[:], scols[:], AX.X, OP.add)
                qsum = sp.tile([P, 1], fp32, name=f"qsum{P}")
                nc.vector.tensor_reduce(qsum[:], qcols[:], AX.X, OP.add)
                mean = sp.tile([P, 1], fp32, name=f"mean{P}")
                nc.vector.tensor_scalar_mul(mean[:], ssum[:], 1.0 / BD)
                ex2 = sp.tile([P, 1], fp32, name=f"ex2{P}")
                nc.vector.tensor_scalar_mul(ex2[:], qsum[:], 1.0 / BD)
                msq = sp.tile([P, 1], fp32, name=f"msq{P}")
                nc.vector.tensor_mul(msq[:], mean[:], mean[:])
                varp = sp.tile([P, 1], fp32, name=f"varp{P}")
                nc.vector.tensor_sub(varp[:], ex2[:], msq[:])
                nc.vector.tensor_scalar_add(varp[:], varp[:], EPS)
                std = sp.tile([P, 1], fp32, name=f"std{P}")
                nc.scalar.sqrt(std[:], varp[:])
                # one Newton step to clean up the scalar-engine sqrt:
                # std' = 0.5*(std + varp/std)
                rstd = sp.tile([P, 1], fp32, name=f"rstd{P}")
                nc.vector.reciprocal(rstd[:], std[:])
                q_ = sp.tile([P, 1], fp32, name=f"q_{P}")
                nc.vector.tensor_mul(q_[:], varp[:], rstd[:])
                nc.vector.tensor_add(std[:], std[:], q_[:])
                nc.vector.tensor_scalar_mul(std[:], std[:], 0.5)
                inv = sp.tile([P, 1], fp32, name=f"inv{P}")
                nc.vector.reciprocal(inv[:], std[:])
                s_ = sp.tile([P, 1], fp32, name=f"s_{P}")
                nc.vector.tensor_mul(s_[:], inv[:], gc[:])
                ms = sp.tile([P, 1], fp32, name=f"ms{P}")
                nc.vector.tensor_mul(ms[:], mean[:], s_[:])
                t_ = sp.tile([P, 1], fp32, name=f"t_{P}")
                nc.vector.tensor_sub(t_[:], bc[:], ms[:])
                return s_, t_

            s1, t1 = bn_finalize(K, s1cols, q1cols, g1c, b1c)
            s0, t0 = bn_finalize(CL, s0cols, q0cols, g0c, b0c)

            # ---- phase 2 ----
            with (
                tc.tile_pool(name="psb", bufs=2, space="PSUM") as psb,  # broadcast
                tc.tile_pool(name="pst", bufs=2, space="PSUM") as pst,  # transposes
                tc.tile_pool(name="psr", bufs=2, space="PSUM") as psr,  # awm+sumA
            ):
                for b in range(B):
                    yb = y1sb[:, b * D : (b + 1) * D]
                    sig1 = wp.tile([K, D], fp32, tag="sig1")
                    nc.scalar.activation(
                        sig1[:], yb, AF.Sigmoid, bias=t1[:], scale=s1[:]
                    )
                    a_t = wp.tile([K, D], fp32, tag="a_t", bufs=3)
                    nc.vector.scalar_tensor_tensor(
                        a_t[:],
                        sig1[:],
                        kmt[:, b : b + 1],
                        kcn[:, b * D : (b + 1) * D],
                        op0=OP.mult,
                        op1=OP.mult,
                    )

                    sig0 = wp.tile([CL, D], fp32, tag="sig0")
                    nc.scalar.activation(
                        sig0[:],
                        y0sb[:, b * D : (b + 1) * D],
                        AF.Sigmoid,
                        bias=t0[:],
                        scale=s0[:],
                    )
                    qg = wp.tile([CL, D], fp32, tag="qg")
                    nc.vector.tensor_scalar_mul(qg[:], sig0[:], cmt[:, b : b + 1])

                    # transposes of sig1 (128x256) and sig0 (16x256) for awm
                    tps = pst.tile([128, 2 * K + 2 * CL], fp32, tag="tps")
                    for h in range(2):
                        nc.tensor.transpose(
                            tps[:, h * K : (h + 1) * K],
                            sig1[:, h * 128 : (h + 1) * 128],
                            iden[:, 0:128],
                        )
                    for h in range(2):
                        nc.tensor.transpose(
                            tps[:, 2 * K + h * CL : 2 * K + (h + 1) * CL],
                            sig0[:, h * 128 : (h + 1) * 128],
                            iden[0:CL, 0:CL],
                        )
                    st = wp.tile([128, 2 * K + 2 * CL], fp32, tag="st")
                    nc.scalar.copy(st[:], tps[:])

                    # awm[c,k] = (1/D) * sum_d sig0T[d,c] * sig1T[d,k]
                    psr_t = psr.tile([CL, K + D], fp32, tag="psr")
                    for h in range(2):
                        nc.tensor.matmul(
                            psr_t[:, 0:K],
                            st[:, 2 * K + h * CL : 2 * K + (h + 1) * CL],
                            st[:, h * K : (h + 1) * K],
                            start=(h == 0),
                            stop=(h == 1),
                        )
                    awm_sb = wp.tile([CL, K], fp32, tag="awm_sb")
                    nc.scalar.mul(awm_sb[:], psr_t[:, 0:K], 1.0 / D)
                    nc.sync.dma_start(awm_d.ap()[b], awm_sb[:])

                    # sumA broadcast to CL partitions: ones(128,CL).T @ A
                    nc.tensor.matmul(
                        psr_t[:, K : K + D], onc[:], a_t[:], start=True, stop=True
                    )
                    av = wp.tile([CL, D], fp32, tag="av")
                    nc.vector.tensor_tensor(av[:], qg[:], psr_t[:, K : K + D], OP.mult)
                    ores_sb = wp.tile([CL, D], fp32, tag="ores_sb")
                    nc.scalar.activation(
                        ores_sb[:], av[:], AF.Tanh, bias=0.0, scale=ilen[:, b : b + 1]
                    )
                    nc.sync.dma_start(ores_d.ap()[b], ores_sb[:])

                    # flatten Qg rows onto one partition for the PE broadcast
                    qgf = wp.tile([1, CL * D], fp32, tag="qgf")
                    nc.sync.dma_start(
                        qgf[0:1].rearrange("o (c d) -> o c d", c=CL), qg[:]
                    )

                    big = bp.tile([K, CL * D], fp32, tag="big")
                    for i in range(4):
                        bc = psb.tile([128, 1024], fp32, tag="bc")
                        for j in range(2):
                            nc.tensor.matmul(
                                bc[:, j * 512 : (j + 1) * 512],
                                onr[:],
                                qgf[0:1, i * 1024 + j * 512 : i * 1024 + (j + 1) * 512],
                                start=True,
                                stop=True,
                            )
                        nc.vector.tensor_tensor(
                            big[:, i * 1024 : (i + 1) * 1024].rearrange(
                                "p (c d) -> p c d", d=D
                            ),
                            bc[:].rearrange("p (c d) -> p c d", d=D),
                            a_t[:].unsqueeze(1).to_broadcast([K, 4, D]),
                            OP.mult,
                        )
                    nc.sync.dma_start(
                        attn_d.ap()[b].rearrange("c k d -> k c d"),
                        big[:].rearrange("p (c d) -> p c d", d=D),
                    )

    nc.compile()
    return nc


def _get_nc():
    if "nc" not in _CACHE:
        _CACHE["nc"] = _build_nc()
    return _CACHE["nc"]


def _make_in_maps(inputs):
    q = np.ascontiguousarray(inputs["query_candidates_repr"], dtype=np.float32)
    kc = np.ascontiguousarray(inputs["key_candidates"], dtype=np.float32)
    W0 = np.asarray(inputs["W0"], dtype=np.float32)
    W1 = np.asarray(inputs["W1"], dtype=np.float32)
    g0 = np.asarray(inputs["bn0_gamma"], dtype=np.float32)
    b0 = np.asarray(inputs["bn0_beta"], dtype=np.float32)
    g1 = np.asarray(inputs["bn1_gamma"], dtype=np.float32)
    b1 = np.asarray(inputs["bn1_beta"], dtype=np.float32)
    cm = np.asarray(inputs["query_candidate_mask"]).astype(np.float32)
    km = np.asarray(inputs["key_candidate_mask"]).astype(np.float32)
    kl = np.asarray(inputs["key_candidate_len"]).astype(np.float32)

    kcn = np.ascontiguousarray(kc.transpose(1, 0, 2))  # (K, B, D)
    kct = np.ascontiguousarray(
        kc.reshape(B, K, 2, 128).transpose(3, 0, 2, 1)
    )  # (128, B, 2, K)
    w0t = np.ascontiguousarray(W0.reshape(D, 2, 128).transpose(2, 1, 0))
    w1t = np.ascontiguousarray(W1.reshape(D, 2, 128).transpose(2, 1, 0))
    kmt = np.ascontiguousarray(km.T)  # (K, B)
    il = np.ascontiguousarray(np.tile(1.0 / kl, (CL, 1)))  # (CL, B)
    iden = np.eye(128, dtype=np.float32)
    onc = np.ones((128, CL), np.float32)
    onr = np.ones((1, 128), np.float32)

    shared = dict(
        kcn=kcn, kct=kct, w0t=w0t, w1t=w1t,
        g1=np.ascontiguousarray(g1.reshape(K, 1)),
        b1=np.ascontiguousarray(b1.reshape(K, 1)),
        kmt=kmt, il=il, iden=iden, onc=onc, onr=onr,
    )
    in_maps = []
    for r in range(NCORES):
        sl = slice(r * CL, (r + 1) * CL)
        qt = np.ascontiguousarray(
            q[:, sl, :].reshape(B, CL, 2, 128).transpose(3, 0, 2, 1)
        )
        m = dict(
            shared,
            qt=qt,
            g0=np.ascontiguousarray(g0[sl].reshape(CL, 1)),
            b0=np.ascontiguousarray(b0[sl].reshape(CL, 1)),
            cmt=np.ascontiguousarray(cm[:, sl].T),
        )
        in_maps.append(m)
    return in_maps


def run(inputs, trace=False):
    from concourse import bass_utils

    nc = _get_nc()
    in_maps = _make_in_maps(inputs)
    res = bass_utils.run_bass_kernel_spmd(
        nc, in_maps, core_ids=list(range(NCORES)), trace=trace
    )
    out_res = np.concatenate([res.results[r]["o_res"] for r in range(NCORES)], axis=1)
    attn = np.concatenate([res.results[r]["o_attn"] for r in range(NCORES)], axis=1)
    awm = np.concatenate([res.results[r]["o_awm"] for r in range(NCORES)], axis=1)
    return (out_res, attn, awm), res


def kernel(**inputs):
    (out_res, attn, awm), _ = run(inputs, trace=False)
    return out_res, attn, awm


# revision 6
# speedup vs baseline: 1.1806x; 1.1806x over previous
"""Trainium2 Bass kernel for nn_AttentionLayer_85383949844589.

Gated attention layer: B=16, C=K=128, D=256.
  g0 = BN0(q @ W0.T)          per-C-channel stats over (B, D)
  g1 = BN1(kc @ W1.T)         per-K-channel stats over (B, D)
  aw[b,c,k,d]   = sigmoid(g1)[b,k,d] * sigmoid(g0)[b,c,d]
  attn[b,c,k,d] = kc[b,k,d] * aw * cmask[b,c] * kmask[b,k]
  out[b,c,d]    = tanh(sum_k attn / klen[b])
  awm[b,c,k]    = mean_d aw

Sharding: the C (query-channel) axis is split across the 8 NeuronCores
(16 channels each).  BN0 stats are per-C-channel, so they are fully local
to a core; the g1/BN1 pipeline is replicated on every core (it is tiny).
No cross-core communication is needed at all.

Per core the dominant cost is writing its (B, C/8, K, D) = 32 MiB slice of
attn, i.e. the kernel is HBM-write-bound (~95 us at ~358 GB/s/core).

Device layouts (host pre-packs everything into DMA-friendly layouts):
  Qg[b]  = sigmoid(g0_local[b]) * cmask      (16 part,  256 free)
  A[b]   = kc[b] * sigmoid(g1[b]) * kmask    (128 part, 256 free)
  attn[b, c, k, :] = Qg[b,c,:] * A[b,k,:]
The row-broadcast of Qg over the 128 k-partitions is done on the PE with a
ones(1,128) stationary matmul into PSUM; the DVE multiplies PSUM by A.
"""

import sys

sys.path.insert(0, "/opt/trn_rl_repo")

import numpy as np

B, C, K, D = 16, 128, 128, 256
NCORES = 8
CL = C // NCORES  # 16 query channels per core
EPS = 1e-5

_CACHE: dict = {}


def _build_nc():
    import concourse.tile as tile
    from concourse import bacc, mybir

    fp32 = mybir.dt.float32
    AF = mybir.ActivationFunctionType
    OP = mybir.AluOpType
    AX = mybir.AxisListType

    nc = bacc.Bacc(trn_type="TRN2", debug=False, num_devices=NCORES)

    # ---- DRAM I/O ----
    # qt[p, b, h, c]  = q[b, c_slice[c], h*128+p]
    qt_d = nc.dram_tensor("qt", [128, B, 2, CL], fp32, kind="ExternalInput")
    # kcn[k, b, d]    = kc[b, k, d]
    kcn_d = nc.dram_tensor("kcn", [K, B, D], fp32, kind="ExternalInput")
    # kct[p, b, h, k] = kc[b, k, h*128+p]
    kct_d = nc.dram_tensor("kct", [128, B, 2, K], fp32, kind="ExternalInput")
    # wXt[p, h, o]    = WX[o, h*128+p]
    w0t_d = nc.dram_tensor("w0t", [128, 2, D], fp32, kind="ExternalInput")
    w1t_d = nc.dram_tensor("w1t", [128, 2, D], fp32, kind="ExternalInput")
    g0_d = nc.dram_tensor("g0", [CL, 1], fp32, kind="ExternalInput")
    b0_d = nc.dram_tensor("b0", [CL, 1], fp32, kind="ExternalInput")
    g1_d = nc.dram_tensor("g1", [K, 1], fp32, kind="ExternalInput")
    b1_d = nc.dram_tensor("b1", [K, 1], fp32, kind="ExternalInput")
    cmt_d = nc.dram_tensor("cmt", [CL, B], fp32, kind="ExternalInput")  # cmask.T
    kmt_d = nc.dram_tensor("kmt", [K, B], fp32, kind="ExternalInput")  # kmask.T
    il_d = nc.dram_tensor("il", [CL, B], fp32, kind="ExternalInput")  # 1/klen
    iden_d = nc.dram_tensor("iden", [128, 128], fp32, kind="ExternalInput")
    onc_d = nc.dram_tensor("onc", [128, CL], fp32, kind="ExternalInput")

    ores_d = nc.dram_tensor("o_res", [B, CL, D], fp32, kind="ExternalOutput")
    attn_d = nc.dram_tensor("o_attn", [B, CL, K, D], fp32, kind="ExternalOutput")
    awm_d = nc.dram_tensor("o_awm", [B, CL, K], fp32, kind="ExternalOutput")

    BD = float(B * D)

    with tile.TileContext(nc) as tc:
        with (
            tc.tile_pool(name="const", bufs=1) as cp,
            tc.tile_pool(name="persist", bufs=1) as pp,
            tc.tile_pool(name="stats", bufs=1) as sp,
            tc.tile_pool(name="work", bufs=2) as wp,
            tc.tile_pool(name="bigout", bufs=3) as bp,
        ):
            # ---- load constants / persistent inputs ----
            w0t = cp.tile([128, 2 * D], fp32)
            nc.sync.dma_start(w0t[:], w0t_d.ap().rearrange("p h o -> p (h o)"))
            w1t = cp.tile([128, 2 * D], fp32)
            nc.sync.dma_start(w1t[:], w1t_d.ap().rearrange("p h o -> p (h o)"))
            iden = cp.tile([128, 128], fp32)
            nc.sync.dma_start(iden[:], iden_d.ap()[:])
            onc = cp.tile([128, CL], fp32)
            nc.sync.dma_start(onc[:], onc_d.ap()[:])
            g0c = cp.tile([CL, 1], fp32)
            nc.sync.dma_start(g0c[:], g0_d.ap()[:])
            b0c = cp.tile([CL, 1], fp32)
            nc.sync.dma_start(b0c[:], b0_d.ap()[:])
            g1c = cp.tile([K, 1], fp32)
            nc.sync.dma_start(g1c[:], g1_d.ap()[:])
            b1c = cp.tile([K, 1], fp32)
            nc.sync.dma_start(b1c[:], b1_d.ap()[:])
            cmt = cp.tile([CL, B], fp32)
            nc.sync.dma_start(cmt[:], cmt_d.ap()[:])
            kmt = cp.tile([K, B], fp32)
            nc.sync.dma_start(kmt[:], kmt_d.ap()[:])
            ilen = cp.tile([CL, B], fp32)
            nc.sync.dma_start(ilen[:], il_d.ap()[:])

            qt = pp.tile([128, B * 2 * CL], fp32)
            nc.sync.dma_start(qt[:], qt_d.ap().rearrange("p b h c -> p (b h c)"))
            kct = pp.tile([128, B * 2 * K], fp32)
            nc.sync.dma_start(kct[:], kct_d.ap().rearrange("p b h k -> p (b h k)"))
            kcn = pp.tile([K, B * D], fp32)
            nc.sync.dma_start(kcn[:], kcn_d.ap().rearrange("k b d -> k (b d)"))

            y1sb = pp.tile([K, B * D], fp32)
            y0sb = pp.tile([CL, B * D], fp32)

            s1cols = sp.tile([K, B], fp32)
            q1cols = sp.tile([K, B], fp32)
            s0cols = sp.tile([CL, B], fp32)
            q0cols = sp.tile([CL, B], fp32)
            sq1s = sp.tile([K, D], fp32)
            sq0s = sp.tile([CL, D], fp32)

            # ---- phase 1: Y0/Y1 matmuls + per-channel sum / sumsq ----
            with tc.tile_pool(name="ps1", bufs=2, space="PSUM") as ps1:
                for b in range(B):
                    y1ps = ps1.tile([K, D], fp32, tag="y1ps")
                    for h in range(2):
                        nc.tensor.matmul(
                            y1ps[:],
                            kct[:, b * 256 + h * 128 : b * 256 + h * 128 + 128],
                            w1t[:, h * D : (h + 1) * D],
                            start=(h == 0),
                            stop=(h == 1),
                        )
                    nc.scalar.activation(
                        y1sb[:, b * D : (b + 1) * D],
                        y1ps[:],
                        AF.Identity,
                        accum_out=s1cols[:, b : b + 1],
                    )
                    nc.scalar.activation(
                        sq1s[:], y1ps[:], AF.Square, accum_out=q1cols[:, b : b + 1]
                    )

                    y0ps = ps1.tile([CL, D], fp32, tag="y0ps")
                    for h in range(2):
                        nc.tensor.matmul(
                            y0ps[:],
                            qt[:, b * 2 * CL + h * CL : b * 2 * CL + (h + 1) * CL],
                            w0t[:, h * D : (h + 1) * D],
                            start=(h == 0),
                            stop=(h == 1),
                        )
                    nc.scalar.activation(
                        y0sb[:, b * D : (b + 1) * D],
                        y0ps[:],
                        AF.Identity,
                        accum_out=s0cols[:, b : b + 1],
                    )
                    nc.scalar.activation(
                        sq0s[:], y0ps[:], AF.Square, accum_out=q0cols[:, b : b + 1]
                    )

            # ---- phase boundary: finalize BN scale/shift ----
            # s = gamma / sqrt(var+eps);  t = beta - mean * s
            def bn_finalize(P, scols, qcols, gc, bc):
                ssum = sp.tile([P, 1], fp32, name=f"ssum{P}")
                nc.vector.tensor_reduce(ssum[:], scols[:], AX.X, OP.add)
                qsum = sp.tile([P, 1], fp32, name=f"qsum{P}")
                nc.vector.tensor_reduce(qsum[:], qcols[:], AX.X, OP.add)
                mean = sp.tile([P, 1], fp32, name=f"mean{P}")
                nc.vector.tensor_scalar_mul(mean[:], ssum[:], 1.0 / BD)
                ex2 = sp.tile([P, 1], fp32, name=f"ex2{P}")
                nc.vector.tensor_scalar_mul(ex2[:], qsum[:], 1.0 / BD)
                msq = sp.tile([P, 1], fp32, name=f"msq{P}")
                nc.vector.tensor_mul(msq[:], mean[:], mean[:])
                varp = sp.tile([P, 1], fp32, name=f"varp{P}")
                nc.vector.tensor_sub(varp[:], ex2[:], msq[:])
                nc.vector.tensor_scalar_add(varp[:], varp[:], EPS)
                std = sp.tile([P, 1], fp32, name=f"std{P}")
                nc.scalar.sqrt(std[:], varp[:])
                # one Newton step to clean up the scalar-engine sqrt:
                # std' = 0.5*(std + varp/std)
                rstd = sp.tile([P, 1], fp32, name=f"rstd{P}")
                nc.vector.reciprocal(rstd[:], std[:])
                q_ = sp.tile([P, 1], fp32, name=f"q_{P}")
                nc.vector.tensor_mul(q_[:], varp[:], rstd[:])
                nc.vector.tensor_add(std[:], std[:], q_[:])
                nc.vector.tensor_scalar_mul(std[:], std[:], 0.5)
                inv = sp.tile([P, 1], fp32, name=f"inv{P}")
                nc.vector.reciprocal(inv[:], std[:])
                s_ = sp.tile([P, 1], fp32, name=f"s_{P}")
                nc.vector.tensor_mul(s_[:], inv[:], gc[:])
                ms = sp.tile([P, 1], fp32, name=f"ms{P}")
                nc.vector.tensor_mul(ms[:], mean[:], s_[:])
                t_ = sp.tile([P, 1], fp32, name=f"t_{P}")
                nc.vector.tensor_sub(t_[:], bc[:], ms[:])
                return s_, t_

            s1, t1 = bn_finalize(K, s1cols, q1cols, g1c, b1c)
            s0, t0 = bn_finalize(CL, s0cols, q0cols, g0c, b0c)

            # ---- phase 2 ----
            with (
                tc.tile_pool(name="pst", bufs=2, space="PSUM") as pst,  # transposes
                tc.tile_pool(name="psr", bufs=2, space="PSUM") as psr,  # awm+sumA
            ):
                for b in range(B):
                    yb = y1sb[:, b * D : (b + 1) * D]
                    sig1 = wp.tile([K, D], fp32, tag="sig1")
                    nc.scalar.activation(
                        sig1[:], yb, AF.Sigmoid, bias=t1[:], scale=s1[:]
                    )
                    a_t = wp.tile([K, D], fp32, tag="a_t", bufs=3)
                    nc.vector.scalar_tensor_tensor(
                        a_t[:],
                        sig1[:],
                        kmt[:, b : b + 1],
                        kcn[:, b * D : (b + 1) * D],
                        op0=OP.mult,
                        op1=OP.mult,
                    )

                    sig0 = wp.tile([CL, D], fp32, tag="sig0")
                    nc.scalar.activation(
                        sig0[:],
                        y0sb[:, b * D : (b + 1) * D],
                        AF.Sigmoid,
                        bias=t0[:],
                        scale=s0[:],
                    )
                    qg = wp.tile([CL, D], fp32, tag="qg")
                    nc.vector.tensor_scalar_mul(qg[:], sig0[:], cmt[:, b : b + 1])

                    # transposes of sig1 (128x256) and sig0 (16x256) for awm
                    tps = pst.tile([128, 2 * K + 2 * CL], fp32, tag="tps")
                    for h in range(2):
                        nc.tensor.transpose(
                            tps[:, h * K : (h + 1) * K],
                            sig1[:, h * 128 : (h + 1) * 128],
                            iden[:, 0:128],
                        )
                    for h in range(2):
                        nc.tensor.transpose(
                            tps[:, 2 * K + h * CL : 2 * K + (h + 1) * CL],
                            sig0[:, h * 128 : (h + 1) * 128],
                            iden[0:CL, 0:CL],
                        )
                    st = wp.tile([128, 2 * K + 2 * CL], fp32, tag="st")
                    nc.scalar.copy(st[:], tps[:])

                    # awm[c,k] = (1/D) * sum_d sig0T[d,c] * sig1T[d,k]
                    psr_t = psr.tile([CL, K + D], fp32, tag="psr")
                    for h in range(2):
                        nc.tensor.matmul(
                            psr_t[:, 0:K],
                            st[:, 2 * K + h * CL : 2 * K + (h + 1) * CL],
                            st[:, h * K : (h + 1) * K],
                            start=(h == 0),
                            stop=(h == 1),
                        )
                    awm_sb = wp.tile([CL, K], fp32, tag="awm_sb")
                    nc.scalar.mul(awm_sb[:], psr_t[:, 0:K], 1.0 / D)
                    nc.sync.dma_start(awm_d.ap()[b], awm_sb[:])

                    # sumA broadcast to CL partitions: ones(128,CL).T @ A
                    nc.tensor.matmul(
                        psr_t[:, K : K + D], onc[:], a_t[:], start=True, stop=True
                    )
                    av = wp.tile([CL, D], fp32, tag="av")
                    nc.vector.tensor_tensor(av[:], qg[:], psr_t[:, K : K + D], OP.mult)
                    ores_sb = wp.tile([CL, D], fp32, tag="ores_sb")
                    nc.scalar.activation(
                        ores_sb[:], av[:], AF.Tanh, bias=0.0, scale=ilen[:, b : b + 1]
                    )
                    nc.sync.dma_start(ores_d.ap()[b], ores_sb[:])

                    # flatten Qg rows onto one partition, then GPSIMD
                    # broadcasts them across all 128 partitions
                    qgf = wp.tile([1, CL * D], fp32, tag="qgf")
                    nc.sync.dma_start(
                        qgf[0:1].rearrange("o (c d) -> o c d", c=CL), qg[:]
                    )
                    bcast = bp.tile([K, CL * D], fp32, tag="bcast", bufs=2)
                    nc.gpsimd.partition_broadcast(bcast[:], qgf[0:1, :])

                    big = bp.tile([K, CL * D], fp32, tag="big")
                    nc.vector.tensor_tensor(
                        big[:].rearrange("p (c d) -> p c d", d=D),
                        bcast[:].rearrange("p (c d) -> p c d", d=D),
                        a_t[:].unsqueeze(1).to_broadcast([K, CL, D]),
                        OP.mult,
                    )
                    nc.sync.dma_start(
                        attn_d.ap()[b].rearrange("c k d -> k c d"),
                        big[:].rearrange("p (c d) -> p c d", d=D),
                    )

    nc.compile()
    return nc


def _get_nc():
    if "nc" not in _CACHE:
        _CACHE["nc"] = _build_nc()
    return _CACHE["nc"]


def _make_in_maps(inputs):
    q = np.ascontiguousarray(inputs["query_candidates_repr"], dtype=np.float32)
    kc = np.ascontiguousarray(inputs["key_candidates"], dtype=np.float32)
    W0 = np.asarray(inputs["W0"], dtype=np.float32)
    W1 = np.asarray(inputs["W1"], dtype=np.float32)
    g0 = np.asarray(inputs["bn0_gamma"], dtype=np.float32)
    b0 = np.asarray(inputs["bn0_beta"], dtype=np.float32)
    g1 = np.asarray(inputs["bn1_gamma"], dtype=np.float32)
    b1 = np.asarray(inputs["bn1_beta"], dtype=np.float32)
    cm = np.asarray(inputs["query_candidate_mask"]).astype(np.float32)
    km = np.asarray(inputs["key_candidate_mask"]).astype(np.float32)
    kl = np.asarray(inputs["key_candidate_len"]).astype(np.float32)

    kcn = np.ascontiguousarray(kc.transpose(1, 0, 2))  # (K, B, D)
    kct = np.ascontiguousarray(
        kc.reshape(B, K, 2, 128).transpose(3, 0, 2, 1)
    )  # (128, B, 2, K)
    w0t = np.ascontiguousarray(W0.reshape(D, 2, 128).transpose(2, 1, 0))
    w1t = np.ascontiguousarray(W1.reshape(D, 2, 128).transpose(2, 1, 0))
    kmt = np.ascontiguousarray(km.T)  # (K, B)
    il = np.ascontiguousarray(np.tile(1.0 / kl, (CL, 1)))  # (CL, B)
    iden = np.eye(128, dtype=np.float32)
    onc = np.ones((128, CL), np.float32)

    shared = dict(
        kcn=kcn, kct=kct, w0t=w0t, w1t=w1t,
        g1=np.ascontiguousarray(g1.reshape(K, 1)),
        b1=np.ascontiguousarray(b1.reshape(K, 1)),
        kmt=kmt, il=il, iden=iden, onc=onc,
    )
    in_maps = []
    for r in range(NCORES):
        sl = slice(r * CL, (r + 1) * CL)
        qt = np.ascontiguousarray(
            q[:, sl, :].reshape(B, CL, 2, 128).transpose(3, 0, 2, 1)
        )
        m = dict(
            shared,
            qt=qt,
            g0=np.ascontiguousarray(g0[sl].reshape(CL, 1)),
            b0=np.ascontiguousarray(b0[sl].reshape(CL, 1)),
            cmt=np.ascontiguousarray(cm[:, sl].T),
        )
        in_maps.append(m)
    return in_maps


def run(inputs, trace=False):
    from concourse import bass_utils

    nc = _get_nc()
    in_maps = _make_in_maps(inputs)
    res = bass_utils.run_bass_kernel_spmd(
        nc, in_maps, core_ids=list(range(NCORES)), trace=trace
    )
    out_res = np.concatenate([res.results[r]["o_res"] for r in range(NCORES)], axis=1)
    attn = np.concatenate([res.results[r]["o_attn"] for r in range(NCORES)], axis=1)
    awm = np.concatenate([res.results[r]["o_awm"] for r in range(NCORES)], axis=1)
    return (out_res, attn, awm), res


def kernel(**inputs):
    (out_res, attn, awm), _ = run(inputs, trace=False)
    return out_res, attn, awm


# revision 12
# speedup vs baseline: 1.4214x; 1.2040x over previous
"""Trainium2 Bass kernel for nn_AttentionLayer_85383949844589.

Gated attention layer: B=16, C=K=128, D=256.
  g0 = BN0(q @ W0.T)          per-C-channel stats over (B, D)
  g1 = BN1(kc @ W1.T)         per-K-channel stats over (B, D)
  aw[b,c,k,d]   = sigmoid(g1)[b,k,d] * sigmoid(g0)[b,c,d]
  attn[b,c,k,d] = kc[b,k,d] * aw * cmask[b,c] * kmask[b,k]
  out[b,c,d]    = tanh(sum_k attn / klen[b])
  awm[b,c,k]    = mean_d aw

Sharding: the C (query-channel) axis is split across the 8 NeuronCores
(16 channels each).  BN0 stats are per-C-channel, so they are fully local
to a core; the g1/BN1 pipeline is replicated on every core (it is tiny).
No cross-core communication is needed at all.

Per core the dominant cost is writing its (B, C/8, K, D) = 32 MiB slice of
attn, i.e. the kernel is HBM-write-bound (~95 us at ~358 GB/s/core).

Device layouts (host pre-packs everything into DMA-friendly layouts):
  Qg[b]  = sigmoid(g0_local[b]) * cmask      (16 part,  256 free)
  A[b]   = kc[b] * sigmoid(g1[b]) * kmask    (128 part, 256 free)
  attn[b, c, k, :] = Qg[b,c,:] * A[b,k,:]
The row-broadcast of Qg over the 128 k-partitions is done on the PE with a
ones(1,128) stationary matmul into PSUM; the DVE multiplies PSUM by A.
"""

import sys

sys.path.insert(0, "/opt/trn_rl_repo")

import numpy as np

B, C, K, D = 16, 128, 128, 256
NCORES = 8
CL = C // NCORES  # 16 query channels per core
EPS = 1e-5

_CACHE: dict = {}


def _build_nc():
    import concourse.tile as tile
    from concourse import bacc, mybir

    fp32 = mybir.dt.float32
    AF = mybir.ActivationFunctionType
    OP = mybir.AluOpType
    AX = mybir.AxisListType

    nc = bacc.Bacc(trn_type="TRN2", debug=False, num_devices=NCORES)

    # ---- DRAM I/O ----
    # qt[p, b, h, c]  = q[b, c_slice[c], h*128+p]
    qt_d = nc.dram_tensor("qt", [128, B, 2, CL], fp32, kind="ExternalInput")
    # kcn[k, b, d]    = kc[b, k, d]
    kcn_d = nc.dram_tensor("kcn", [K, B, D], fp32, kind="ExternalInput")
    # kct[p, b, h, k] = kc[b, k, h*128+p]
    kct_d = nc.dram_tensor("kct", [128, B, 2, K], fp32, kind="ExternalInput")
    # wXt[p, h, o]    = WX[o, h*128+p]
    w0t_d = nc.dram_tensor("w0t", [128, 2, D], fp32, kind="ExternalInput")
    w1t_d = nc.dram_tensor("w1t", [128, 2, D], fp32, kind="ExternalInput")
    g0_d = nc.dram_tensor("g0", [CL, 1], fp32, kind="ExternalInput")
    b0_d = nc.dram_tensor("b0", [CL, 1], fp32, kind="ExternalInput")
    g1_d = nc.dram_tensor("g1", [K, 1], fp32, kind="ExternalInput")
    b1_d = nc.dram_tensor("b1", [K, 1], fp32, kind="ExternalInput")
    cmt_d = nc.dram_tensor("cmt", [CL, B], fp32, kind="ExternalInput")  # cmask.T
    kmt_d = nc.dram_tensor("kmt", [K, B], fp32, kind="ExternalInput")  # kmask.T
    il_d = nc.dram_tensor("il", [CL, B], fp32, kind="ExternalInput")  # 1/klen
    iden_d = nc.dram_tensor("iden", [128, 128], fp32, kind="ExternalInput")
    onc_d = nc.dram_tensor("onc", [128, CL], fp32, kind="ExternalInput")

    ores_d = nc.dram_tensor("o_res", [B, CL, D], fp32, kind="ExternalOutput")
    attn_d = nc.dram_tensor("o_attn", [B, CL, K, D], fp32, kind="ExternalOutput")
    awm_d = nc.dram_tensor("o_awm", [B, CL, K], fp32, kind="ExternalOutput")

    BD = float(B * D)

    with tile.TileContext(nc) as tc:
        with (
            tc.tile_pool(name="const", bufs=1) as cp,
            tc.tile_pool(name="persist", bufs=1) as pp,
            tc.tile_pool(name="stats", bufs=1) as sp,
            tc.tile_pool(name="work", bufs=2) as wp,
            tc.tile_pool(name="bigout", bufs=3) as bp,
        ):
            # ---- load constants / persistent inputs ----
            w0t = cp.tile([128, 2 * D], fp32)
            nc.sync.dma_start(w0t[:], w0t_d.ap().rearrange("p h o -> p (h o)"))
            w1t = cp.tile([128, 2 * D], fp32)
            nc.sync.dma_start(w1t[:], w1t_d.ap().rearrange("p h o -> p (h o)"))
            iden = cp.tile([128, 128], fp32)
            nc.sync.dma_start(iden[:], iden_d.ap()[:])
            onc = cp.tile([128, CL], fp32)
            nc.sync.dma_start(onc[:], onc_d.ap()[:])
            g0c = cp.tile([CL, 1], fp32)
            nc.sync.dma_start(g0c[:], g0_d.ap()[:])
            b0c = cp.tile([CL, 1], fp32)
            nc.sync.dma_start(b0c[:], b0_d.ap()[:])
            g1c = cp.tile([K, 1], fp32)
            nc.sync.dma_start(g1c[:], g1_d.ap()[:])
            b1c = cp.tile([K, 1], fp32)
            nc.sync.dma_start(b1c[:], b1_d.ap()[:])
            cmt = cp.tile([CL, B], fp32)
            nc.sync.dma_start(cmt[:], cmt_d.ap()[:])
            kmt = cp.tile([K, B], fp32)
            nc.sync.dma_start(kmt[:], kmt_d.ap()[:])
            ilen = cp.tile([CL, B], fp32)
            nc.sync.dma_start(ilen[:], il_d.ap()[:])

            qt = pp.tile([128, B * 2 * CL], fp32)
            nc.sync.dma_start(qt[:], qt_d.ap().rearrange("p b h c -> p (b h c)"))
            kct = pp.tile([128, B * 2 * K], fp32)
            nc.sync.dma_start(kct[:], kct_d.ap().rearrange("p b h k -> p (b h k)"))
            kcn = pp.tile([K, B * D], fp32)
            nc.sync.dma_start(kcn[:], kcn_d.ap().rearrange("k b d -> k (b d)"))

            y1sb = pp.tile([K, B * D], fp32)
            y0sb = pp.tile([CL, B * D], fp32)

            s1cols = sp.tile([K, B], fp32)
            q1cols = sp.tile([K, B], fp32)
            s0cols = sp.tile([CL, B], fp32)
            q0cols = sp.tile([CL, B], fp32)
            sq1s = sp.tile([K, D], fp32)
            sq0s = sp.tile([CL, D], fp32)

            # ---- phase 1: Y0/Y1 matmuls + per-channel sum / sumsq ----
            with tc.tile_pool(name="ps1", bufs=2, space="PSUM") as ps1:
                for b in range(B):
                    y1ps = ps1.tile([K, D], fp32, tag="y1ps")
                    for h in range(2):
                        nc.tensor.matmul(
                            y1ps[:],
                            kct[:, b * 256 + h * 128 : b * 256 + h * 128 + 128],
                            w1t[:, h * D : (h + 1) * D],
                            start=(h == 0),
                            stop=(h == 1),
                        )
                    nc.scalar.activation(
                        y1sb[:, b * D : (b + 1) * D],
                        y1ps[:],
                        AF.Identity,
                        accum_out=s1cols[:, b : b + 1],
                    )
                    nc.scalar.activation(
                        sq1s[:], y1ps[:], AF.Square, accum_out=q1cols[:, b : b + 1]
                    )

                    y0ps = ps1.tile([CL, D], fp32, tag="y0ps")
                    for h in range(2):
                        nc.tensor.matmul(
                            y0ps[:],
                            qt[:, b * 2 * CL + h * CL : b * 2 * CL + (h + 1) * CL],
                            w0t[:, h * D : (h + 1) * D],
                            start=(h == 0),
                            stop=(h == 1),
                        )
                    nc.scalar.activation(
                        y0sb[:, b * D : (b + 1) * D],
                        y0ps[:],
                        AF.Identity,
                        accum_out=s0cols[:, b : b + 1],
                    )
                    nc.scalar.activation(
                        sq0s[:], y0ps[:], AF.Square, accum_out=q0cols[:, b : b + 1]
                    )

            # ---- phase boundary: finalize BN scale/shift ----
            # s = gamma / sqrt(var+eps);  t = beta - mean * s
            def bn_finalize(P, scols, qcols, gc, bc):
                ssum = sp.tile([P, 1], fp32, name=f"ssum{P}")
                nc.vector.tensor_reduce(ssum[:], scols[:], AX.X, OP.add)
                qsum = sp.tile([P, 1], fp32, name=f"qsum{P}")
                nc.vector.tensor_reduce(qsum[:], qcols[:], AX.X, OP.add)
                mean = sp.tile([P, 1], fp32, name=f"mean{P}")
                nc.vector.tensor_scalar_mul(mean[:], ssum[:], 1.0 / BD)
                ex2 = sp.tile([P, 1], fp32, name=f"ex2{P}")
                nc.vector.tensor_scalar_mul(ex2[:], qsum[:], 1.0 / BD)
                msq = sp.tile([P, 1], fp32, name=f"msq{P}")
                nc.vector.tensor_mul(msq[:], mean[:], mean[:])
                varp = sp.tile([P, 1], fp32, name=f"varp{P}")
                nc.vector.tensor_sub(varp[:], ex2[:], msq[:])
                nc.vector.tensor_scalar_add(varp[:], varp[:], EPS)
                std = sp.tile([P, 1], fp32, name=f"std{P}")
                nc.scalar.sqrt(std[:], varp[:])
                # one Newton step to clean up the scalar-engine sqrt:
                # std' = 0.5*(std + varp/std)
                rstd = sp.tile([P, 1], fp32, name=f"rstd{P}")
                nc.vector.reciprocal(rstd[:], std[:])
                q_ = sp.tile([P, 1], fp32, name=f"q_{P}")
                nc.vector.tensor_mul(q_[:], varp[:], rstd[:])
                nc.vector.tensor_add(std[:], std[:], q_[:])
                nc.vector.tensor_scalar_mul(std[:], std[:], 0.5)
                inv = sp.tile([P, 1], fp32, name=f"inv{P}")
                nc.vector.reciprocal(inv[:], std[:])
                s_ = sp.tile([P, 1], fp32, name=f"s_{P}")
                nc.vector.tensor_mul(s_[:], inv[:], gc[:])
                ms = sp.tile([P, 1], fp32, name=f"ms{P}")
                nc.vector.tensor_mul(ms[:], mean[:], s_[:])
                t_ = sp.tile([P, 1], fp32, name=f"t_{P}")
                nc.vector.tensor_sub(t_[:], bc[:], ms[:])
                return s_, t_

            s1, t1 = bn_finalize(K, s1cols, q1cols, g1c, b1c)
            s0, t0 = bn_finalize(CL, s0cols, q0cols, g0c, b0c)

            # Bake cmask into a per-(c,b) scale/bias so Qg = sigmoid-masked
            # comes straight off the scalar engine:
            #   masked: sigmoid(s0*y + t0);  unmasked: sigmoid(0*y - 1e30) = 0
            s0b = sp.tile([CL, B], fp32)
            nc.vector.tensor_scalar(s0b[:], cmt[:], s0[:], None, OP.mult)
            t0b = sp.tile([CL, B], fp32)
            # t0b = t0*cm + (cm-1)*1e30
            nc.vector.tensor_scalar(t0b[:], cmt[:], 1.0, 1e30, OP.subtract, OP.mult)
            tb2 = sp.tile([CL, B], fp32)
            nc.vector.tensor_scalar(tb2[:], cmt[:], t0[:], None, OP.mult)
            nc.vector.tensor_add(t0b[:], t0b[:], tb2[:])

            # ---- phase 2 ----
            with (
                tc.tile_pool(name="pst", bufs=2, space="PSUM") as pst,  # transposes
                tc.tile_pool(name="psr", bufs=2, space="PSUM") as psr,  # awm+sumA
            ):
                for b in range(B):
                    # alternate HWDGE queues so the big output DMA never
                    # head-of-line-blocks the small pipeline DMAs
                    dq = nc.sync if (b % 2 == 0) else nc.scalar
                    oq = nc.scalar if (b % 2 == 0) else nc.sync

                    yb = y1sb[:, b * D : (b + 1) * D]
                    sig1 = wp.tile([K, D], fp32, tag="sig1")
                    nc.scalar.activation(
                        sig1[:], yb, AF.Sigmoid, bias=t1[:], scale=s1[:]
                    )
                    a_t = wp.tile([K, D], fp32, tag="a_t", bufs=3)
                    nc.vector.scalar_tensor_tensor(
                        a_t[:],
                        sig1[:],
                        kmt[:, b : b + 1],
                        kcn[:, b * D : (b + 1) * D],
                        op0=OP.mult,
                        op1=OP.mult,
                    )

                    sig0 = wp.tile([CL, D], fp32, tag="sig0")
                    nc.scalar.activation(
                        sig0[:],
                        y0sb[:, b * D : (b + 1) * D],
                        AF.Sigmoid,
                        bias=t0[:],
                        scale=s0[:],
                    )
                    qg = wp.tile([CL, D], fp32, tag="qg")
                    nc.scalar.activation(
                        qg[:],
                        y0sb[:, b * D : (b + 1) * D],
                        AF.Sigmoid,
                        bias=t0b[:, b : b + 1],
                        scale=s0b[:, b : b + 1],
                    )

                    # transposes of sig1 (128x256) and sig0 (16x256) for awm
                    tps = pst.tile([128, 2 * K + 2 * CL], fp32, tag="tps")
                    for h in range(2):
                        nc.tensor.transpose(
                            tps[:, h * K : (h + 1) * K],
                            sig1[:, h * 128 : (h + 1) * 128],
                            iden[:, 0:128],
                        )
                    for h in range(2):
                        nc.tensor.transpose(
                            tps[:, 2 * K + h * CL : 2 * K + (h + 1) * CL],
                            sig0[:, h * 128 : (h + 1) * 128],
                            iden[0:CL, 0:CL],
                        )
                    st = wp.tile([128, 2 * K + 2 * CL], fp32, tag="st")
                    nc.scalar.copy(st[:], tps[:])

                    # awm[c,k] = (1/D) * sum_d sig0T[d,c] * sig1T[d,k]
                    psr_t = psr.tile([CL, K + D], fp32, tag="psr")
                    for h in range(2):
                        nc.tensor.matmul(
                            psr_t[:, 0:K],
                            st[:, 2 * K + h * CL : 2 * K + (h + 1) * CL],
                            st[:, h * K : (h + 1) * K],
                            start=(h == 0),
                            stop=(h == 1),
                        )
                    awm_sb = wp.tile([CL, K], fp32, tag="awm_sb")
                    nc.scalar.mul(awm_sb[:], psr_t[:, 0:K], 1.0 / D)
                    oq.dma_start(awm_d.ap()[b], awm_sb[:])

                    # sumA broadcast to CL partitions: ones(128,CL).T @ A
                    nc.tensor.matmul(
                        psr_t[:, K : K + D], onc[:], a_t[:], start=True, stop=True
                    )
                    av = wp.tile([CL, D], fp32, tag="av")
                    nc.vector.tensor_tensor(av[:], qg[:], psr_t[:, K : K + D], OP.mult)
                    ores_sb = wp.tile([CL, D], fp32, tag="ores_sb")
                    nc.scalar.activation(
                        ores_sb[:], av[:], AF.Tanh, bias=0.0, scale=ilen[:, b : b + 1]
                    )
                    oq.dma_start(ores_d.ap()[b], ores_sb[:])

                    # flatten Qg rows onto one partition, then GPSIMD
                    # broadcasts them across all 128 partitions
                    qgf = wp.tile([1, CL * D], fp32, tag="qgf")
                    oq.dma_start(
                        qgf[0:1].rearrange("o (c d) -> o c d", c=CL), qg[:]
                    )
                    bcast = bp.tile([K, CL * D], fp32, tag="bcast", bufs=2)
                    nc.gpsimd.partition_broadcast(bcast[:], qgf[0:1, :])

                    big = bp.tile([K, CL * D], fp32, tag="big")
                    nc.vector.tensor_tensor(
                        big[:].rearrange("p (c d) -> p c d", d=D),
                        bcast[:].rearrange("p (c d) -> p c d", d=D),
                        a_t[:].unsqueeze(1).to_broadcast([K, CL, D]),
                        OP.mult,
                    )
                    dq.dma_start(
                        attn_d.ap()[b].rearrange("c k d -> k c d"),
                        big[:].rearrange("p (c d) -> p c d", d=D),
                    )

    nc.compile()
    return nc


def _get_nc():
    if "nc" not in _CACHE:
        _CACHE["nc"] = _build_nc()
    return _CACHE["nc"]


def _make_in_maps(inputs):
    q = np.ascontiguousarray(inputs["query_candidates_repr"], dtype=np.float32)
    kc = np.ascontiguousarray(inputs["key_candidates"], dtype=np.float32)
    W0 = np.asarray(inputs["W0"], dtype=np.float32)
    W1 = np.asarray(inputs["W1"], dtype=np.float32)
    g0 = np.asarray(inputs["bn0_gamma"], dtype=np.float32)
    b0 = np.asarray(inputs["bn0_beta"], dtype=np.float32)
    g1 = np.asarray(inputs["bn1_gamma"], dtype=np.float32)
    b1 = np.asarray(inputs["bn1_beta"], dtype=np.float32)
    cm = np.asarray(inputs["query_candidate_mask"]).astype(np.float32)
    km = np.asarray(inputs["key_candidate_mask"]).astype(np.float32)
    kl = np.asarray(inputs["key_candidate_len"]).astype(np.float32)

    kcn = np.ascontiguousarray(kc.transpose(1, 0, 2))  # (K, B, D)
    kct = np.ascontiguousarray(
        kc.reshape(B, K, 2, 128).transpose(3, 0, 2, 1)
    )  # (128, B, 2, K)
    w0t = np.ascontiguousarray(W0.reshape(D, 2, 128).transpose(2, 1, 0))
    w1t = np.ascontiguousarray(W1.reshape(D, 2, 128).transpose(2, 1, 0))
    kmt = np.ascontiguousarray(km.T)  # (K, B)
    il = np.ascontiguousarray(np.tile(1.0 / kl, (CL, 1)))  # (CL, B)
    iden = np.eye(128, dtype=np.float32)
    onc = np.ones((128, CL), np.float32)

    shared = dict(
        kcn=kcn, kct=kct, w0t=w0t, w1t=w1t,
        g1=np.ascontiguousarray(g1.reshape(K, 1)),
        b1=np.ascontiguousarray(b1.reshape(K, 1)),
        kmt=kmt, il=il, iden=iden, onc=onc,
    )
    in_maps = []
    for r in range(NCORES):
        sl = slice(r * CL, (r + 1) * CL)
        qt = np.ascontiguousarray(
            q[:, sl, :].reshape(B, CL, 2, 128).transpose(3, 0, 2, 1)
        )
        m = dict(
            shared,
            qt=qt,
            g0=np.ascontiguousarray(g0[sl].reshape(CL, 1)),
            b0=np.ascontiguousarray(b0[sl].reshape(CL, 1)),
            cmt=np.ascontiguousarray(cm[:, sl].T),
        )
        in_maps.append(m)
    return in_maps


def run(inputs, trace=False):
    from concourse import bass_utils

    nc = _get_nc()
    in_maps = _make_in_maps(inputs)
    res = bass_utils.run_bass_kernel_spmd(
        nc, in_maps, core_ids=list(range(NCORES)), trace=trace
    )
    out_res = np.concatenate([res.results[r]["o_res"] for r in range(NCORES)], axis=1)
    attn = np.concatenate([res.results[r]["o_attn"] for r in range(NCORES)], axis=1)
    awm = np.concatenate([res.results[r]["o_awm"] for r in range(NCORES)], axis=1)
    return (out_res, attn, awm), res


def kernel(**inputs):
    (out_res, attn, awm), _ = run(inputs, trace=False)
    return out_res, attn, awm


# revision 19
# speedup vs baseline: 2.0664x; 1.4538x over previous
"""Trainium2 Bass kernel for nn_AttentionLayer_85383949844589.

Gated attention layer: B=16, C=K=128, D=256.
  g0 = BN0(q @ W0.T)          per-C-channel stats over (B, D)
  g1 = BN1(kc @ W1.T)         per-K-channel stats over (B, D)
  aw[b,c,k,d]   = sigmoid(g1)[b,k,d] * sigmoid(g0)[b,c,d]
  attn[b,c,k,d] = kc[b,k,d] * aw * cmask[b,c] * kmask[b,k]
  out[b,c,d]    = tanh(sum_k attn / klen[b])
  awm[b,c,k]    = mean_d aw

Sharding: the C (query-channel) axis is split across the 8 NeuronCores
(16 channels each).  BN0 stats are per-C-channel, so they are fully local
to a core; the g1/BN1 pipeline is replicated on every core (it is tiny).
No cross-core communication is needed at all.

Per core the dominant cost is writing its (B, C/8, K, D) = 32 MiB slice of
attn, i.e. the kernel is HBM-write-bound (~95 us at ~358 GB/s/core).

Device layouts (host pre-packs everything into DMA-friendly layouts):
  Qg[b]  = sigmoid(g0_local[b]) * cmask      (16 part,  256 free)
  A[b]   = kc[b] * sigmoid(g1[b]) * kmask    (128 part, 256 free)
  attn[b, c, k, :] = Qg[b,c,:] * A[b,k,:]
The row-broadcast of Qg over the 128 k-partitions is done on the PE with a
ones(1,128) stationary matmul into PSUM; the DVE multiplies PSUM by A.
"""

import sys

sys.path.insert(0, "/opt/trn_rl_repo")

import numpy as np

B, C, K, D = 16, 128, 128, 256
NCORES = 8
CL = C // NCORES  # 16 query channels per core
EPS = 1e-5

_CACHE: dict = {}


def _build_nc():
    import concourse.tile as tile
    from concourse import bacc, mybir

    fp32 = mybir.dt.float32
    AF = mybir.ActivationFunctionType
    OP = mybir.AluOpType
    AX = mybir.AxisListType

    nc = bacc.Bacc(trn_type="TRN2", debug=False, num_devices=NCORES)

    # ---- DRAM I/O ----
    # qt[p, b, h, c]  = q[b, c_slice[c], h*128+p]
    qt_d = nc.dram_tensor("qt", [128, B, 2, CL], fp32, kind="ExternalInput")
    # kcn[k, b, d]    = kc[b, k, d]
    kcn_d = nc.dram_tensor("kcn", [K, B, D], fp32, kind="ExternalInput")
    # kct[p, b, h, k] = kc[b, k, h*128+p]
    kct_d = nc.dram_tensor("kct", [128, B, 2, K], fp32, kind="ExternalInput")
    # wXt[p, h, o]    = WX[o, h*128+p]
    w0t_d = nc.dram_tensor("w0t", [128, 2, D], fp32, kind="ExternalInput")
    w1t_d = nc.dram_tensor("w1t", [128, 2, D], fp32, kind="ExternalInput")
    g0_d = nc.dram_tensor("g0", [CL, 1], fp32, kind="ExternalInput")
    b0_d = nc.dram_tensor("b0", [CL, 1], fp32, kind="ExternalInput")
    g1_d = nc.dram_tensor("g1", [K, 1], fp32, kind="ExternalInput")
    b1_d = nc.dram_tensor("b1", [K, 1], fp32, kind="ExternalInput")
    cmt_d = nc.dram_tensor("cmt", [CL, B], fp32, kind="ExternalInput")  # cmask.T
    kmt_d = nc.dram_tensor("kmt", [K, B], fp32, kind="ExternalInput")  # kmask.T
    il_d = nc.dram_tensor("il", [128, B], fp32, kind="ExternalInput")  # 1/klen
    iden_d = nc.dram_tensor("iden", [128, 128], fp32, kind="ExternalInput")

    # transposed layouts (d on partitions); host reassembles
    ores_d = nc.dram_tensor("o_res", [B, 2, 128, CL], fp32, kind="ExternalOutput")
    attn_d = nc.dram_tensor("o_attn", [B, 2, 128, CL, K], fp32, kind="ExternalOutput")
    awm_d = nc.dram_tensor("o_awm", [B, CL, K], fp32, kind="ExternalOutput")

    BD = float(B * D)

    with tile.TileContext(nc) as tc:
        with (
            tc.tile_pool(name="const", bufs=1) as cp,
            tc.tile_pool(name="persist", bufs=1) as pp,
            tc.tile_pool(name="stats", bufs=1) as sp,
            tc.tile_pool(name="work", bufs=2) as wp,
            tc.tile_pool(name="bigout", bufs=3) as bp,
        ):
            # ---- load constants / persistent inputs ----
            w0t = cp.tile([128, 2 * D], fp32)
            nc.sync.dma_start(w0t[:], w0t_d.ap().rearrange("p h o -> p (h o)"))
            w1t = cp.tile([128, 2 * D], fp32)
            nc.sync.dma_start(w1t[:], w1t_d.ap().rearrange("p h o -> p (h o)"))
            iden = cp.tile([128, 128], fp32)
            nc.sync.dma_start(iden[:], iden_d.ap()[:])
            g0c = cp.tile([CL, 1], fp32)
            nc.sync.dma_start(g0c[:], g0_d.ap()[:])
            b0c = cp.tile([CL, 1], fp32)
            nc.sync.dma_start(b0c[:], b0_d.ap()[:])
            g1c = cp.tile([K, 1], fp32)
            nc.sync.dma_start(g1c[:], g1_d.ap()[:])
            b1c = cp.tile([K, 1], fp32)
            nc.sync.dma_start(b1c[:], b1_d.ap()[:])
            cmt = cp.tile([CL, B], fp32)
            nc.sync.dma_start(cmt[:], cmt_d.ap()[:])
            kmt = cp.tile([K, B], fp32)
            nc.sync.dma_start(kmt[:], kmt_d.ap()[:])
            ilen = cp.tile([128, B], fp32)
            nc.sync.dma_start(ilen[:], il_d.ap()[:])

            qt = pp.tile([128, B * 2 * CL], fp32)
            nc.sync.dma_start(qt[:], qt_d.ap().rearrange("p b h c -> p (b h c)"))
            kct = pp.tile([128, B * 2 * K], fp32)
            nc.sync.dma_start(kct[:], kct_d.ap().rearrange("p b h k -> p (b h k)"))
            kcn = pp.tile([K, B * D], fp32)
            nc.sync.dma_start(kcn[:], kcn_d.ap().rearrange("k b d -> k (b d)"))

            y1sb = pp.tile([K, B * D], fp32)
            y0sb = pp.tile([CL, B * D], fp32)

            s1cols = sp.tile([K, B], fp32)
            q1cols = sp.tile([K, B], fp32)
            s0cols = sp.tile([CL, B], fp32)
            q0cols = sp.tile([CL, B], fp32)
            sq1s = sp.tile([K, D], fp32)
            sq0s = sp.tile([CL, D], fp32)

            # ---- phase 1: Y0/Y1 matmuls + per-channel sum / sumsq ----
            with tc.tile_pool(name="ps1", bufs=2, space="PSUM") as ps1:
                for b in range(B):
                    y1ps = ps1.tile([K, D], fp32, tag="y1ps")
                    for h in range(2):
                        nc.tensor.matmul(
                            y1ps[:],
                            kct[:, b * 256 + h * 128 : b * 256 + h * 128 + 128],
                            w1t[:, h * D : (h + 1) * D],
                            start=(h == 0),
                            stop=(h == 1),
                        )
                    nc.scalar.activation(
                        y1sb[:, b * D : (b + 1) * D],
                        y1ps[:],
                        AF.Identity,
                        accum_out=s1cols[:, b : b + 1],
                    )
                    nc.scalar.activation(
                        sq1s[:], y1ps[:], AF.Square, accum_out=q1cols[:, b : b + 1]
                    )

                    y0ps = ps1.tile([CL, D], fp32, tag="y0ps")
                    for h in range(2):
                        nc.tensor.matmul(
                            y0ps[:],
                            qt[:, b * 2 * CL + h * CL : b * 2 * CL + (h + 1) * CL],
                            w0t[:, h * D : (h + 1) * D],
                            start=(h == 0),
                            stop=(h == 1),
                        )
                    nc.scalar.activation(
                        y0sb[:, b * D : (b + 1) * D],
                        y0ps[:],
                        AF.Identity,
                        accum_out=s0cols[:, b : b + 1],
                    )
                    nc.scalar.activation(
                        sq0s[:], y0ps[:], AF.Square, accum_out=q0cols[:, b : b + 1]
                    )

            # ---- phase boundary: finalize BN scale/shift ----
            # s = gamma / sqrt(var+eps);  t = beta - mean * s
            def bn_finalize(P, scols, qcols, gc, bc):
                ssum = sp.tile([P, 1], fp32, name=f"ssum{P}")
                nc.vector.tensor_reduce(ssum[:], scols[:], AX.X, OP.add)
                qsum = sp.tile([P, 1], fp32, name=f"qsum{P}")
                nc.vector.tensor_reduce(qsum[:], qcols[:], AX.X, OP.add)
                mean = sp.tile([P, 1], fp32, name=f"mean{P}")
                nc.vector.tensor_scalar_mul(mean[:], ssum[:], 1.0 / BD)
                ex2 = sp.tile([P, 1], fp32, name=f"ex2{P}")
                nc.vector.tensor_scalar_mul(ex2[:], qsum[:], 1.0 / BD)
                msq = sp.tile([P, 1], fp32, name=f"msq{P}")
                nc.vector.tensor_mul(msq[:], mean[:], mean[:])
                varp = sp.tile([P, 1], fp32, name=f"varp{P}")
                nc.vector.tensor_sub(varp[:], ex2[:], msq[:])
                nc.vector.tensor_scalar_add(varp[:], varp[:], EPS)
                std = sp.tile([P, 1], fp32, name=f"std{P}")
                nc.scalar.sqrt(std[:], varp[:])
                # one Newton step to clean up the scalar-engine sqrt:
                # std' = 0.5*(std + varp/std)
                rstd = sp.tile([P, 1], fp32, name=f"rstd{P}")
                nc.vector.reciprocal(rstd[:], std[:])
                q_ = sp.tile([P, 1], fp32, name=f"q_{P}")
                nc.vector.tensor_mul(q_[:], varp[:], rstd[:])
                nc.vector.tensor_add(std[:], std[:], q_[:])
                nc.vector.tensor_scalar_mul(std[:], std[:], 0.5)
                inv = sp.tile([P, 1], fp32, name=f"inv{P}")
                nc.vector.reciprocal(inv[:], std[:])
                s_ = sp.tile([P, 1], fp32, name=f"s_{P}")
                nc.vector.tensor_mul(s_[:], inv[:], gc[:])
                ms = sp.tile([P, 1], fp32, name=f"ms{P}")
                nc.vector.tensor_mul(ms[:], mean[:], s_[:])
                t_ = sp.tile([P, 1], fp32, name=f"t_{P}")
                nc.vector.tensor_sub(t_[:], bc[:], ms[:])
                return s_, t_

            s1, t1 = bn_finalize(K, s1cols, q1cols, g1c, b1c)
            s0, t0 = bn_finalize(CL, s0cols, q0cols, g0c, b0c)

            # Bake cmask into a per-(c,b) scale/bias so Qg = sigmoid-masked
            # comes straight off the scalar engine:
            #   masked: sigmoid(s0*y + t0);  unmasked: sigmoid(0*y - 1e30) = 0
            s0b = sp.tile([CL, B], fp32)
            nc.vector.tensor_scalar(s0b[:], cmt[:], s0[:], None, OP.mult)
            t0b = sp.tile([CL, B], fp32)
            # t0b = t0*cm + (cm-1)*1e30
            nc.vector.tensor_scalar(t0b[:], cmt[:], 1.0, 1e30, OP.subtract, OP.mult)
            tb2 = sp.tile([CL, B], fp32)
            nc.vector.tensor_scalar(tb2[:], cmt[:], t0[:], None, OP.mult)
            nc.vector.tensor_add(t0b[:], t0b[:], tb2[:])

            # ---- phase 2 ----
            # PSUM transpose-staging layout, all within bank boundaries:
            #   [  0:128) sig1T h0   [128:256) sig1T h1     (bank 0)
            #   [256:384) A_t  h0                            (bank 0)
            #   [384:400) sig0T h0   [400:416) sig0T h1     (bank 0)
            #   [416:432) QgT  h0    [432:448) QgT  h1      (bank 0)
            #   [512:640) A_t  h1                            (bank 1)
            S1T, AT0, S0T, QGT, AT1 = 0, 256, 384, 416, 512
            with (
                tc.tile_pool(name="pst", bufs=2, space="PSUM") as pst,  # transposes
                tc.tile_pool(name="psr", bufs=2, space="PSUM") as psr,  # awm
            ):
                for b in range(B):
                    # alternate HWDGE queues so the big output DMA never
                    # head-of-line-blocks the small pipeline DMAs
                    dq = nc.sync if (b % 2 == 0) else nc.scalar
                    oq = nc.scalar if (b % 2 == 0) else nc.sync

                    yb = y1sb[:, b * D : (b + 1) * D]
                    sig1 = wp.tile([K, D], fp32, tag="sig1")
                    nc.scalar.activation(
                        sig1[:], yb, AF.Sigmoid, bias=t1[:], scale=s1[:]
                    )
                    a_t = wp.tile([K, D], fp32, tag="a_t", bufs=3)
                    nc.vector.scalar_tensor_tensor(
                        a_t[:],
                        sig1[:],
                        kmt[:, b : b + 1],
                        kcn[:, b * D : (b + 1) * D],
                        op0=OP.mult,
                        op1=OP.mult,
                    )

                    sig0 = wp.tile([CL, D], fp32, tag="sig0")
                    nc.scalar.activation(
                        sig0[:],
                        y0sb[:, b * D : (b + 1) * D],
                        AF.Sigmoid,
                        bias=t0[:],
                        scale=s0[:],
                    )
                    qg = wp.tile([CL, D], fp32, tag="qg")
                    nc.scalar.activation(
                        qg[:],
                        y0sb[:, b * D : (b + 1) * D],
                        AF.Sigmoid,
                        bias=t0b[:, b : b + 1],
                        scale=s0b[:, b : b + 1],
                    )

                    # transpose everything into d-on-partitions layout
                    tps = pst.tile([128, 640], fp32, tag="tps")
                    for h in range(2):
                        nc.tensor.transpose(
                            tps[:, S1T + h * K : S1T + (h + 1) * K],
                            sig1[:, h * 128 : (h + 1) * 128],
                            iden[:, 0:128],
                        )
                        nc.tensor.transpose(
                            tps[:, (AT0, AT1)[h] : (AT0, AT1)[h] + K],
                            a_t[:, h * 128 : (h + 1) * 128],
                            iden[:, 0:128],
                        )
                        nc.tensor.transpose(
                            tps[:, S0T + h * CL : S0T + (h + 1) * CL],
                            sig0[:, h * 128 : (h + 1) * 128],
                            iden[0:CL, 0:CL],
                        )
                        nc.tensor.transpose(
                            tps[:, QGT + h * CL : QGT + (h + 1) * CL],
                            qg[:, h * 128 : (h + 1) * 128],
                            iden[0:CL, 0:CL],
                        )
                    st = wp.tile([128, 640], fp32, tag="st")
                    nc.scalar.copy(st[:], tps[:])

                    # awm[c,k] = (1/D) * sum_d sig0T[d,c] * sig1T[d,k]
                    psr_t = psr.tile([CL, K], fp32, tag="psr")
                    for h in range(2):
                        nc.tensor.matmul(
                            psr_t[:],
                            st[:, S0T + h * CL : S0T + (h + 1) * CL],
                            st[:, S1T + h * K : S1T + (h + 1) * K],
                            start=(h == 0),
                            stop=(h == 1),
                        )
                    awm_sb = wp.tile([CL, K], fp32, tag="awm_sb")
                    nc.scalar.mul(awm_sb[:], psr_t[:], 1.0 / D)
                    oq.dma_start(awm_d.ap()[b], awm_sb[:])

                    # attention_vector (transposed): sum_k A_t along free,
                    # then av_t[d,c] = QgT[d,c] * sumA[d], tanh(av/klen)
                    sA = wp.tile([128, 2], fp32, tag="sA")
                    av_t = wp.tile([128, 2 * CL], fp32, tag="av_t")
                    for h in range(2):
                        nc.vector.tensor_reduce(
                            sA[:, h : h + 1],
                            st[:, (AT0, AT1)[h] : (AT0, AT1)[h] + K],
                            AX.X,
                            OP.add,
                        )
                        nc.vector.tensor_scalar_mul(
                            av_t[:, h * CL : (h + 1) * CL],
                            st[:, QGT + h * CL : QGT + (h + 1) * CL],
                            sA[:, h : h + 1],
                        )
                    ores_t = wp.tile([128, 2 * CL], fp32, tag="ores_t")
                    nc.scalar.activation(
                        ores_t[:], av_t[:], AF.Tanh, bias=0.0, scale=ilen[:, b : b + 1]
                    )
                    oq.dma_start(
                        ores_d.ap()[b].rearrange("h p c -> p h c"),
                        ores_t[:].rearrange("p (h c) -> p h c", c=CL),
                    )

                    # big product, d on partitions:
                    #   big_t[d, c, k] = QgT[d, c] * A_t[d, k]
                    big = bp.tile([128, 2 * CL * K], fp32, tag="big")
                    for h in range(2):
                        nc.vector.tensor_tensor(
                            big[:, h * CL * K : (h + 1) * CL * K].rearrange(
                                "p (c k) -> p c k", k=K
                            ),
                            st[:, QGT + h * CL : QGT + (h + 1) * CL]
                            .unsqueeze(2)
                            .to_broadcast([128, CL, K]),
                            st[:, (AT0, AT1)[h] : (AT0, AT1)[h] + K]
                            .unsqueeze(1)
                            .to_broadcast([128, CL, K]),
                            OP.mult,
                        )
                    dq.dma_start(
                        attn_d.ap()[b].rearrange("h p c k -> p h c k"),
                        big[:].rearrange("p (h c k) -> p h c k", c=CL, k=K),
                    )

    nc.compile()
    return nc


def _get_nc():
    if "nc" not in _CACHE:
        _CACHE["nc"] = _build_nc()
    return _CACHE["nc"]


def _make_in_maps(inputs):
    q = np.ascontiguousarray(inputs["query_candidates_repr"], dtype=np.float32)
    kc = np.ascontiguousarray(inputs["key_candidates"], dtype=np.float32)
    W0 = np.asarray(inputs["W0"], dtype=np.float32)
    W1 = np.asarray(inputs["W1"], dtype=np.float32)
    g0 = np.asarray(inputs["bn0_gamma"], dtype=np.float32)
    b0 = np.asarray(inputs["bn0_beta"], dtype=np.float32)
    g1 = np.asarray(inputs["bn1_gamma"], dtype=np.float32)
    b1 = np.asarray(inputs["bn1_beta"], dtype=np.float32)
    cm = np.asarray(inputs["query_candidate_mask"]).astype(np.float32)
    km = np.asarray(inputs["key_candidate_mask"]).astype(np.float32)
    kl = np.asarray(inputs["key_candidate_len"]).astype(np.float32)

    kcn = np.ascontiguousarray(kc.transpose(1, 0, 2))  # (K, B, D)
    kct = np.ascontiguousarray(
        kc.reshape(B, K, 2, 128).transpose(3, 0, 2, 1)
    )  # (128, B, 2, K)
    w0t = np.ascontiguousarray(W0.reshape(D, 2, 128).transpose(2, 1, 0))
    w1t = np.ascontiguousarray(W1.reshape(D, 2, 128).transpose(2, 1, 0))
    kmt = np.ascontiguousarray(km.T)  # (K, B)
    il = np.ascontiguousarray(np.tile(1.0 / kl, (128, 1)))  # (128, B)
    iden = np.eye(128, dtype=np.float32)

    shared = dict(
        kcn=kcn, kct=kct, w0t=w0t, w1t=w1t,
        g1=np.ascontiguousarray(g1.reshape(K, 1)),
        b1=np.ascontiguousarray(b1.reshape(K, 1)),
        kmt=kmt, il=il, iden=iden,
    )
    in_maps = []
    for r in range(NCORES):
        sl = slice(r * CL, (r + 1) * CL)
        qt = np.ascontiguousarray(
            q[:, sl, :].reshape(B, CL, 2, 128).transpose(3, 0, 2, 1)
        )
        m = dict(
            shared,
            qt=qt,
            g0=np.ascontiguousarray(g0[sl].reshape(CL, 1)),
            b0=np.ascontiguousarray(b0[sl].reshape(CL, 1)),
            cmt=np.ascontiguousarray(cm[:, sl].T),
        )
        in_maps.append(m)
    return in_maps


def run(inputs, trace=False):
    from concourse import bass_utils

    nc = _get_nc()
    in_maps = _make_in_maps(inputs)
    res = bass_utils.run_bass_kernel_spmd(
        nc, in_maps, core_ids=list(range(NCORES)), trace=trace
    )
    # device outputs are d-on-partitions (B, 2, 128, CL[, K]); restore layout
    ores_t = np.stack([res.results[r]["o_res"] for r in range(NCORES)], axis=3)
    # (B, 2, 128, NCORES, CL) -> (B, C, D)
    out_res = np.ascontiguousarray(
        ores_t.transpose(0, 3, 4, 1, 2).reshape(B, C, D)
    )
    attn_t = np.stack([res.results[r]["o_attn"] for r in range(NCORES)], axis=3)
    # (B, 2, 128, NCORES, CL, K) -> (B, C, K, D)
    attn = np.ascontiguousarray(
        attn_t.transpose(0, 3, 4, 5, 1, 2).reshape(B, C, K, D)
    )
    awm = np.concatenate([res.results[r]["o_awm"] for r in range(NCORES)], axis=1)
    return (out_res, attn, awm), res


def kernel(**inputs):
    (out_res, attn, awm), _ = run(inputs, trace=False)
    return out_res, attn, awm


# revision 32
# speedup vs baseline: 2.2190x; 1.0739x over previous
"""Trainium2 Bass kernel for nn_AttentionLayer_85383949844589.

Gated attention layer: B=16, C=K=128, D=256.
  g0 = BN0(q @ W0.T)          per-C-channel stats over (B, D)
  g1 = BN1(kc @ W1.T)         per-K-channel stats over (B, D)
  aw[b,c,k,d]   = sigmoid(g1)[b,k,d] * sigmoid(g0)[b,c,d]
  attn[b,c,k,d] = kc[b,k,d] * aw * cmask[b,c] * kmask[b,k]
  out[b,c,d]    = tanh(sum_k attn / klen[b])
  awm[b,c,k]    = mean_d aw

Sharding: the C (query-channel) axis is split across the 8 NeuronCores
(16 channels each).  BN0 stats are per-C-channel, so they are fully local
to a core; the g1/BN1 pipeline is replicated on every core (it is tiny).
No cross-core communication is needed at all.

Per core the dominant cost is writing its (B, C/8, K, D) = 32 MiB slice of
attn, i.e. the kernel is HBM-write-bound (~95 us at ~358 GB/s/core).

Device layouts (host pre-packs everything into DMA-friendly layouts):
  Qg[b]  = sigmoid(g0_local[b]) * cmask      (16 part,  256 free)
  A[b]   = kc[b] * sigmoid(g1[b]) * kmask    (128 part, 256 free)
  attn[b, c, k, :] = Qg[b,c,:] * A[b,k,:]
The row-broadcast of Qg over the 128 k-partitions is done on the PE with a
ones(1,128) stationary matmul into PSUM; the DVE multiplies PSUM by A.
"""

import sys

sys.path.insert(0, "/opt/trn_rl_repo")

import numpy as np

B, C, K, D = 16, 128, 128, 256
NCORES = 8
CL = C // NCORES  # 16 query channels per core
EPS = 1e-5

_CACHE: dict = {}


def _build_nc():
    import concourse.tile as tile
    from concourse import bacc, mybir

    fp32 = mybir.dt.float32
    AF = mybir.ActivationFunctionType
    OP = mybir.AluOpType
    AX = mybir.AxisListType

    nc = bacc.Bacc(trn_type="TRN2", debug=False, num_devices=NCORES)

    # ---- DRAM I/O ----
    # qt[p, b, h, c]  = q[b, c_slice[c], h*128+p]
    qt_d = nc.dram_tensor("qt", [128, B, 2, CL], fp32, kind="ExternalInput")
    # kct[p, b, h, k] = kc[b, k, h*128+p]
    kct_d = nc.dram_tensor("kct", [128, B, 2, K], fp32, kind="ExternalInput")
    # kcm[p, b, h, k] = kc[b, k, h*128+p] * kmask[b, k]
    kcm_d = nc.dram_tensor("kcm", [128, B, 2, K], fp32, kind="ExternalInput")
    # wXt[p, h, o]    = WX[o, h*128+p]
    w0t_d = nc.dram_tensor("w0t", [128, 2, D], fp32, kind="ExternalInput")
    w1t_d = nc.dram_tensor("w1t", [128, 2, D], fp32, kind="ExternalInput")
    g0_d = nc.dram_tensor("g0", [CL, 1], fp32, kind="ExternalInput")
    b0_d = nc.dram_tensor("b0", [CL, 1], fp32, kind="ExternalInput")
    g1_d = nc.dram_tensor("g1", [K, 1], fp32, kind="ExternalInput")
    b1_d = nc.dram_tensor("b1", [K, 1], fp32, kind="ExternalInput")
    cmt_d = nc.dram_tensor("cmt", [CL, B], fp32, kind="ExternalInput")  # cmask.T
    il_d = nc.dram_tensor("il", [128, B], fp32, kind="ExternalInput")  # 1/klen
    iden_d = nc.dram_tensor("iden", [128, 128], fp32, kind="ExternalInput")

    # transposed layouts (d on partitions); host reassembles
    ores_d = nc.dram_tensor("o_res", [B, 2, 128, CL], fp32, kind="ExternalOutput")
    attn_d = nc.dram_tensor("o_attn", [B, 2, 128, CL, K], fp32, kind="ExternalOutput")
    awm_d = nc.dram_tensor("o_awm", [B, CL, K], fp32, kind="ExternalOutput")

    BD = float(B * D)

    with tile.TileContext(nc) as tc:
        with (
            tc.tile_pool(name="const", bufs=1) as cp,
            tc.tile_pool(name="persist", bufs=1) as pp,
            tc.tile_pool(name="stats", bufs=1) as sp,
            tc.tile_pool(name="work", bufs=2) as wp,
            tc.tile_pool(name="bigout", bufs=3) as bp,
        ):
            # ---- load constants / persistent inputs ----
            w0t = cp.tile([128, 2 * D], fp32)
            nc.sync.dma_start(w0t[:], w0t_d.ap().rearrange("p h o -> p (h o)"))
            w1t = cp.tile([128, 2 * D], fp32)
            nc.sync.dma_start(w1t[:], w1t_d.ap().rearrange("p h o -> p (h o)"))
            iden = cp.tile([128, 128], fp32)
            nc.sync.dma_start(iden[:], iden_d.ap()[:])
            g0c = cp.tile([CL, 1], fp32)
            nc.sync.dma_start(g0c[:], g0_d.ap()[:])
            b0c = cp.tile([CL, 1], fp32)
            nc.sync.dma_start(b0c[:], b0_d.ap()[:])
            g1c = cp.tile([K, 1], fp32)
            nc.sync.dma_start(g1c[:], g1_d.ap()[:])
            b1c = cp.tile([K, 1], fp32)
            nc.sync.dma_start(b1c[:], b1_d.ap()[:])
            cmt = cp.tile([CL, B], fp32)
            nc.sync.dma_start(cmt[:], cmt_d.ap()[:])
            ilen = cp.tile([128, B], fp32)
            nc.sync.dma_start(ilen[:], il_d.ap()[:])

            qt = pp.tile([128, B * 2 * CL], fp32)
            nc.sync.dma_start(qt[:], qt_d.ap().rearrange("p b h c -> p (b h c)"))
            # chunked so the first Y1 matmuls start after ~1/4 of the load
            kct = pp.tile([128, B * 2 * K], fp32)
            kcm = pp.tile([128, B * 2 * K], fp32)
            CH = B // 4
            for i in range(4):
                eng = nc.sync if i % 2 == 0 else nc.scalar
                eng.dma_start(
                    kct[:, i * CH * 2 * K : (i + 1) * CH * 2 * K],
                    kct_d.ap()[:, i * CH : (i + 1) * CH].rearrange(
                        "p b h k -> p (b h k)"
                    ),
                )
                eng.dma_start(
                    kcm[:, i * CH * 2 * K : (i + 1) * CH * 2 * K],
                    kcm_d.ap()[:, i * CH : (i + 1) * CH].rearrange(
                        "p b h k -> p (b h k)"
                    ),
                )

            y1sb = pp.tile([K, B * D], fp32)
            y0sb = pp.tile([CL, B * D], fp32)

            s1cols = sp.tile([K, B], fp32)
            q1cols = sp.tile([K, B], fp32)
            s0cols = sp.tile([CL, B], fp32)
            q0cols = sp.tile([CL, B], fp32)
            sq1s = sp.tile([K, D], fp32)
            sq0s = sp.tile([CL, D], fp32)

            # ---- phase 1: Y0/Y1 matmuls + per-channel sum / sumsq ----
            with tc.tile_pool(name="ps1", bufs=2, space="PSUM") as ps1:
                for b in range(B):
                    y1ps = ps1.tile([K, D], fp32, tag="y1ps")
                    for h in range(2):
                        nc.tensor.matmul(
                            y1ps[:],
                            kct[:, b * 256 + h * 128 : b * 256 + h * 128 + 128],
                            w1t[:, h * D : (h + 1) * D],
                            start=(h == 0),
                            stop=(h == 1),
                        )
                    nc.scalar.activation(
                        y1sb[:, b * D : (b + 1) * D],
                        y1ps[:],
                        AF.Identity,
                        accum_out=s1cols[:, b : b + 1],
                    )
                    nc.scalar.activation(
                        sq1s[:], y1ps[:], AF.Square, accum_out=q1cols[:, b : b + 1]
                    )

                    y0ps = ps1.tile([CL, D], fp32, tag="y0ps")
                    for h in range(2):
                        nc.tensor.matmul(
                            y0ps[:],
                            qt[:, b * 2 * CL + h * CL : b * 2 * CL + (h + 1) * CL],
                            w0t[:, h * D : (h + 1) * D],
                            start=(h == 0),
                            stop=(h == 1),
                        )
                    nc.scalar.activation(
                        y0sb[:, b * D : (b + 1) * D],
                        y0ps[:],
                        AF.Identity,
                        accum_out=s0cols[:, b : b + 1],
                    )
                    nc.scalar.activation(
                        sq0s[:], y0ps[:], AF.Square, accum_out=q0cols[:, b : b + 1]
                    )

            # ---- phase boundary: finalize BN scale/shift ----
            # s = gamma / sqrt(var+eps);  t = beta - mean * s
            def bn_finalize(P, scols, qcols, gc, bc):
                ssum = sp.tile([P, 1], fp32, name=f"ssum{P}")
                nc.vector.tensor_reduce(ssum[:], scols[:], AX.X, OP.add)
                qsum = sp.tile([P, 1], fp32, name=f"qsum{P}")
                nc.vector.tensor_reduce(qsum[:], qcols[:], AX.X, OP.add)
                mean = sp.tile([P, 1], fp32, name=f"mean{P}")
                nc.vector.tensor_scalar_mul(mean[:], ssum[:], 1.0 / BD)
                ex2 = sp.tile([P, 1], fp32, name=f"ex2{P}")
                nc.vector.tensor_scalar_mul(ex2[:], qsum[:], 1.0 / BD)
                msq = sp.tile([P, 1], fp32, name=f"msq{P}")
                nc.vector.tensor_mul(msq[:], mean[:], mean[:])
                varp = sp.tile([P, 1], fp32, name=f"varp{P}")
                nc.vector.tensor_sub(varp[:], ex2[:], msq[:])
                nc.vector.tensor_scalar_add(varp[:], varp[:], EPS)
                std = sp.tile([P, 1], fp32, name=f"std{P}")
                nc.scalar.sqrt(std[:], varp[:])
                # one Newton step to clean up the scalar-engine sqrt:
                # std' = 0.5*(std + varp/std)
                rstd = sp.tile([P, 1], fp32, name=f"rstd{P}")
                nc.vector.reciprocal(rstd[:], std[:])
                q_ = sp.tile([P, 1], fp32, name=f"q_{P}")
                nc.vector.tensor_mul(q_[:], varp[:], rstd[:])
                nc.vector.tensor_add(std[:], std[:], q_[:])
                nc.vector.tensor_scalar_mul(std[:], std[:], 0.5)
                inv = sp.tile([P, 1], fp32, name=f"inv{P}")
                nc.vector.reciprocal(inv[:], std[:])
                s_ = sp.tile([P, 1], fp32, name=f"s_{P}")
                nc.vector.tensor_mul(s_[:], inv[:], gc[:])
                ms = sp.tile([P, 1], fp32, name=f"ms{P}")
                nc.vector.tensor_mul(ms[:], mean[:], s_[:])
                t_ = sp.tile([P, 1], fp32, name=f"t_{P}")
                nc.vector.tensor_sub(t_[:], bc[:], ms[:])
                return s_, t_

            s1, t1 = bn_finalize(K, s1cols, q1cols, g1c, b1c)
            s0, t0 = bn_finalize(CL, s0cols, q0cols, g0c, b0c)

            # Bake cmask into a per-(c,b) scale/bias so Qg = sigmoid-masked
            # comes straight off the scalar engine:
            #   masked: sigmoid(s0*y + t0);  unmasked: sigmoid(0*y - 1e30) = 0
            s0b = sp.tile([CL, B], fp32)
            nc.vector.tensor_scalar(s0b[:], cmt[:], s0[:], None, OP.mult)
            t0b = sp.tile([CL, B], fp32)
            # t0b = t0*cm + (cm-1)*1e30
            nc.vector.tensor_scalar(t0b[:], cmt[:], 1.0, 1e30, OP.subtract, OP.mult)
            tb2 = sp.tile([CL, B], fp32)
            nc.vector.tensor_scalar(tb2[:], cmt[:], t0[:], None, OP.mult)
            nc.vector.tensor_add(t0b[:], t0b[:], tb2[:])

            # ---- phase 2 ----
            # PSUM transpose-staging layout (single bank):
            #   [  0:128) sig1T h0   [128:256) sig1T h1
            #   [256:272) sig0T h0   [272:288) sig0T h1
            #   [288:304) QgT  h0    [304:320) QgT  h1
            S1T, S0T, QGT = 0, 256, 288
            with (
                tc.tile_pool(name="pst", bufs=2, space="PSUM") as pst,  # transposes
                tc.tile_pool(name="psr", bufs=2, space="PSUM") as psr,  # awm
            ):
                for b in range(B):
                    # alternate HWDGE queues so the big output DMA never
                    # head-of-line-blocks the small pipeline DMAs
                    dq = nc.sync if (b % 2 == 0) else nc.scalar
                    oq = nc.scalar if (b % 2 == 0) else nc.sync

                    yb = y1sb[:, b * D : (b + 1) * D]
                    sig1 = wp.tile([K, D], fp32, tag="sig1")
                    nc.scalar.activation(
                        sig1[:], yb, AF.Sigmoid, bias=t1[:], scale=s1[:]
                    )

                    sig0 = wp.tile([CL, D], fp32, tag="sig0")
                    nc.scalar.activation(
                        sig0[:],
                        y0sb[:, b * D : (b + 1) * D],
                        AF.Sigmoid,
                        bias=t0[:],
                        scale=s0[:],
                    )
                    qg = wp.tile([CL, D], fp32, tag="qg")
                    nc.scalar.activation(
                        qg[:],
                        y0sb[:, b * D : (b + 1) * D],
                        AF.Sigmoid,
                        bias=t0b[:, b : b + 1],
                        scale=s0b[:, b : b + 1],
                    )

                    # transpose into d-on-partitions layout
                    tps = pst.tile([128, 320], fp32, tag="tps")
                    for h in range(2):
                        nc.tensor.transpose(
                            tps[:, S1T + h * K : S1T + (h + 1) * K],
                            sig1[:, h * 128 : (h + 1) * 128],
                            iden[:, 0:128],
                        )
                        nc.tensor.transpose(
                            tps[:, S0T + h * CL : S0T + (h + 1) * CL],
                            sig0[:, h * 128 : (h + 1) * 128],
                            iden[0:CL, 0:CL],
                        )
                        nc.tensor.transpose(
                            tps[:, QGT + h * CL : QGT + (h + 1) * CL],
                            qg[:, h * 128 : (h + 1) * 128],
                            iden[0:CL, 0:CL],
                        )
                    st = wp.tile([128, 320], fp32, tag="st")
                    nc.scalar.copy(st[:], tps[:])

                    # A_t[d, k] = sig1T[d, k] * (kc*kmask)T[d, k]
                    at2 = wp.tile([128, 2 * K], fp32, tag="at2")
                    for h in range(2):
                        nc.vector.tensor_tensor(
                            at2[:, h * K : (h + 1) * K],
                            st[:, S1T + h * K : S1T + (h + 1) * K],
                            kcm[:, b * 2 * K + h * K : b * 2 * K + (h + 1) * K],
                            OP.mult,
                        )

                    # awm[c,k] = (1/D) * sum_d sig0T[d,c] * sig1T[d,k]
                    psr_t = psr.tile([CL, K], fp32, tag="psr")
                    for h in range(2):
                        nc.tensor.matmul(
                            psr_t[:],
                            st[:, S0T + h * CL : S0T + (h + 1) * CL],
                            st[:, S1T + h * K : S1T + (h + 1) * K],
                            start=(h == 0),
                            stop=(h == 1),
                        )
                    awm_sb = wp.tile([CL, K], fp32, tag="awm_sb")
                    nc.scalar.mul(awm_sb[:], psr_t[:], 1.0 / D)
                    oq.dma_start(awm_d.ap()[b], awm_sb[:])

                    # attention_vector (transposed): sum_k A_t along free,
                    # then av_t[d,c] = QgT[d,c] * sumA[d], tanh(av/klen)
                    sA = wp.tile([128, 2], fp32, tag="sA")
                    av_t = wp.tile([128, 2 * CL], fp32, tag="av_t")
                    for h in range(2):
                        nc.vector.tensor_reduce(
                            sA[:, h : h + 1],
                            at2[:, h * K : (h + 1) * K],
                            AX.X,
                            OP.add,
                        )
                        nc.vector.tensor_scalar_mul(
                            av_t[:, h * CL : (h + 1) * CL],
                            st[:, QGT + h * CL : QGT + (h + 1) * CL],
                            sA[:, h : h + 1],
                        )
                    ores_t = wp.tile([128, 2 * CL], fp32, tag="ores_t")
                    nc.scalar.activation(
                        ores_t[:], av_t[:], AF.Tanh, bias=0.0, scale=ilen[:, b : b + 1]
                    )
                    oq.dma_start(
                        ores_d.ap()[b].rearrange("h p c -> p h c"),
                        ores_t[:].rearrange("p (h c) -> p h c", c=CL),
                    )

                    # big product, d on partitions:
                    #   big_t[d, c, k] = QgT[d, c] * A_t[d, k]
                    big = bp.tile([128, 2 * CL * K], fp32, tag="big")
                    for h in range(2):
                        nc.vector.tensor_tensor(
                            big[:, h * CL * K : (h + 1) * CL * K].rearrange(
                                "p (c k) -> p c k", k=K
                            ),
                            st[:, QGT + h * CL : QGT + (h + 1) * CL]
                            .unsqueeze(2)
                            .to_broadcast([128, CL, K]),
                            at2[:, h * K : (h + 1) * K]
                            .unsqueeze(1)
                            .to_broadcast([128, CL, K]),
                            OP.mult,
                        )
                    dq.dma_start(
                        attn_d.ap()[b].rearrange("h p c k -> p h c k"),
                        big[:].rearrange("p (h c k) -> p h c k", c=CL, k=K),
                    )

    nc.compile()
    return nc


def _get_nc():
    if "nc" not in _CACHE:
        _CACHE["nc"] = _build_nc()
    return _CACHE["nc"]


def _make_in_maps(inputs):
    q = np.ascontiguousarray(inputs["query_candidates_repr"], dtype=np.float32)
    kc = np.ascontiguousarray(inputs["key_candidates"], dtype=np.float32)
    W0 = np.asarray(inputs["W0"], dtype=np.float32)
    W1 = np.asarray(inputs["W1"], dtype=np.float32)
    g0 = np.asarray(inputs["bn0_gamma"], dtype=np.float32)
    b0 = np.asarray(inputs["bn0_beta"], dtype=np.float32)
    g1 = np.asarray(inputs["bn1_gamma"], dtype=np.float32)
    b1 = np.asarray(inputs["bn1_beta"], dtype=np.float32)
    cm = np.asarray(inputs["query_candidate_mask"]).astype(np.float32)
    km = np.asarray(inputs["key_candidate_mask"]).astype(np.float32)
    kl = np.asarray(inputs["key_candidate_len"]).astype(np.float32)

    kct = np.ascontiguousarray(
        kc.reshape(B, K, 2, 128).transpose(3, 0, 2, 1)
    )  # (128, B, 2, K)
    kcm = np.ascontiguousarray(
        (kc * km[:, :, None]).reshape(B, K, 2, 128).transpose(3, 0, 2, 1)
    )  # (128, B, 2, K), kmask folded in
    w0t = np.ascontiguousarray(W0.reshape(D, 2, 128).transpose(2, 1, 0))
    w1t = np.ascontiguousarray(W1.reshape(D, 2, 128).transpose(2, 1, 0))
    il = np.ascontiguousarray(np.tile(1.0 / kl, (128, 1)))  # (128, B)
    iden = np.eye(128, dtype=np.float32)

    shared = dict(
        kct=kct, kcm=kcm, w0t=w0t, w1t=w1t,
        g1=np.ascontiguousarray(g1.reshape(K, 1)),
        b1=np.ascontiguousarray(b1.reshape(K, 1)),
        il=il, iden=iden,
    )
    in_maps = []
    for r in range(NCORES):
        sl = slice(r * CL, (r + 1) * CL)
        qt = np.ascontiguousarray(
            q[:, sl, :].reshape(B, CL, 2, 128).transpose(3, 0, 2, 1)
        )
        m = dict(
            shared,
            qt=qt,
            g0=np.ascontiguousarray(g0[sl].reshape(CL, 1)),
            b0=np.ascontiguousarray(b0[sl].reshape(CL, 1)),
            cmt=np.ascontiguousarray(cm[:, sl].T),
        )
        in_maps.append(m)
    return in_maps


def run(inputs, trace=False):
    from concourse import bass_utils

    nc = _get_nc()
    in_maps = _make_in_maps(inputs)
    res = bass_utils.run_bass_kernel_spmd(
        nc, in_maps, core_ids=list(range(NCORES)), trace=trace
    )
    # device outputs are d-on-partitions (B, 2, 128, CL[, K]); restore layout
    ores_t = np.stack([res.results[r]["o_res"] for r in range(NCORES)], axis=3)
    # (B, 2, 128, NCORES, CL) -> (B, C, D)
    out_res = np.ascontiguousarray(
        ores_t.transpose(0, 3, 4, 1, 2).reshape(B, C, D)
    )
    attn_t = np.stack([res.results[r]["o_attn"] for r in range(NCORES)], axis=3)
    # (B, 2, 128, NCORES, CL, K) -> (B, C, K, D)
    attn = np.ascontiguousarray(
        attn_t.transpose(0, 3, 4, 5, 1, 2).reshape(B, C, K, D)
    )
    awm = np.concatenate([res.results[r]["o_awm"] for r in range(NCORES)], axis=1)
    return (out_res, attn, awm), res


def kernel(**inputs):
    (out_res, attn, awm), _ = run(inputs, trace=False)
    return out_res, attn, awm
